# revision 14
# baseline (speedup 1.0000x reference)
"""Two-layer GCN (PyG GCNConv semantics) on 8 Trainium2 NeuronCores.

Strategy (graph/data parallel, dst-sharded):
  - Nodes are sharded row-wise across the 8 cores (12500 each, padded to
    12544 = 98*128). Weights are replicated.
  - Phase 1: each core computes its shard of h = x @ W1 (bf16 matmul).
  - Phase 2: AllGather h shards -> full bf16 node-feature table per core.
  - Phase 3: per-core aggregation over incoming edges of its dst shard:
    dma_gather of h[src] rows + one-hot scaled selection matmuls that
    scatter-accumulate into PSUM per 128-dst tile; + b1, ReLU,
    then hr @ W2 (via PE transpose) -> per-node 16-wide g vectors.
  - Phase 4: AllGather g shards -> full g table.
  - Phase 5: same aggregation structure over g rows + b2 -> output shard.

Edge preprocessing (host, numpy): symmetric-norm coefficients, self loops,
edges sorted/grouped per (core, tile-batch, src-window, tile) with
capacities fixed to the max across cores so all 8 cores run one SPMD NEFF.
Index tables are int16 (hardware gather constraint) relative to 25088-row
windows of the padded node table.
"""

import math

import ml_dtypes
import numpy as np

import concourse.bacc as bacc
import concourse.bass as bass
import concourse.mybir as mybir
import concourse.tile as tile
from concourse.bass_utils import run_bass_kernel_spmd
from concourse.library_config import mlp as mlp_lib

F32 = mybir.dt.float32
BF16 = mybir.dt.bfloat16
I16 = mybir.dt.int16
BF = ml_dtypes.bfloat16


class Cfg:
    def __init__(self, N=100000, E=3200000, FIN=512, HID=256, FOUT=16,
                 NCORES=8, TB=3, SLAB_CH=8, MAX_WIN=32768):
        self.N, self.E, self.FIN, self.HID, self.FOUT = N, E, FIN, HID, FOUT
        self.NCORES, self.TB, self.SLAB_CH = NCORES, TB, SLAB_CH
        assert N % NCORES == 0
        self.SH_RAW = N // NCORES
        self.TPS = (self.SH_RAW + 127) // 128          # tiles per shard
        self.SH = self.TPS * 128                        # padded shard rows
        self.PN = NCORES * self.SH                      # padded table rows
        self.W_SH = max(1, min(MAX_WIN // self.SH, NCORES))
        while NCORES % self.W_SH:
            self.W_SH -= 1
        self.WIN = self.W_SH * self.SH                  # window rows (int16-addressable)
        assert self.WIN <= 32768
        self.NWIN = NCORES // self.W_SH
        self.NB = (self.TPS + TB - 1) // TB             # tile batches
        assert FIN % 128 == 0 and HID % 128 == 0 and FOUT <= 128


class Plan:
    """Static (core-independent) edge-stream structure."""

    def __init__(self, cfg, cap):
        # cap: [TPS, NWIN] slot capacity (multiples of 128)
        self.cfg = cfg
        self.cap = cap
        self.batches = [list(range(b * cfg.TB, min((b + 1) * cfg.TB, cfg.TPS)))
                        for b in range(cfg.NB)]
        # per (t, w) chunk offset in the global stream (in chunks of 128 slots)
        self.group_choff = np.zeros((cfg.TPS, cfg.NWIN), np.int64)
        self.batch_choff = []           # chunk offset of each batch
        self.batch_nch = []             # chunks in each batch
        self.batch_slabs = []           # [(w, c0, c1)] chunk ranges (batch-rel)
        self.batch_chunk_tiles = []     # per-chunk tile id
        self.batch_first = []           # tile -> first batch-rel chunk
        self.batch_last = []            # tile -> last batch-rel chunk
        off = 0
        for b, tiles in enumerate(self.batches):
            self.batch_choff.append(off)
            ctiles = []
            slabs = []
            first, last = {}, {}
            for w in range(cfg.NWIN):
                w0 = len(ctiles)
                for t in tiles:
                    self.group_choff[t, w] = off + len(ctiles)
                    nch = cap[t, w] // 128
                    for _ in range(nch):
                        first.setdefault(t, len(ctiles))
                        last[t] = len(ctiles)
                        ctiles.append(t)
                # split into pieces of at most SLAB_CH chunks (each gather's
                # descriptor count must fit the SWDGE ring with headroom;
                # <= SLAB_CH distinct sizes keeps num_idxs register use low)
                c0 = w0
                rem = len(ctiles) - w0
                while rem:
                    p = min(cfg.SLAB_CH, rem)
                    slabs.append((w, c0, c0 + p))
                    c0 += p
                    rem -= p
            self.batch_chunk_tiles.append(ctiles)
            self.batch_slabs.append(slabs)
            self.batch_first.append(first)
            self.batch_last.append(last)
            self.batch_nch.append(len(ctiles))
            off += len(ctiles)
        self.total_ch = off
        self.L = off * 128
        self.max_batch_ch = max(self.batch_nch)


def _preprocess(cfg, x, edge_index, edge_weight, W1, b1, W2, b2):
    N, NC = cfg.N, cfg.NCORES
    src = np.asarray(edge_index[0]).astype(np.int64)
    dst = np.asarray(edge_index[1]).astype(np.int64)
    ew = np.asarray(edge_weight).astype(np.float32)

    # self loops (weight 1.0), symmetric normalization at dst
    deg = np.bincount(dst, weights=ew.astype(np.float64), minlength=N) + 1.0
    dinv = (1.0 / np.sqrt(deg)).astype(np.float32)
    src_f = np.concatenate([src, np.arange(N, dtype=np.int64)])
    dst_f = np.concatenate([dst, np.arange(N, dtype=np.int64)])
    ew_f = np.concatenate([ew, np.ones(N, np.float32)])
    norm = dinv[src_f] * ew_f * dinv[dst_f]

    core = dst_f // cfg.SH_RAW
    dl = dst_f % cfg.SH_RAW
    t = dl // 128
    dtl = (dl % 128).astype(np.float32)
    rsrc = (src_f // cfg.SH_RAW) * cfg.SH + (src_f % cfg.SH_RAW)
    w = rsrc // cfg.WIN
    widx = (rsrc % cfg.WIN).astype(np.int16)

    cnt = np.bincount((core * cfg.TPS + t) * cfg.NWIN + w,
                      minlength=NC * cfg.TPS * cfg.NWIN
                      ).reshape(NC, cfg.TPS, cfg.NWIN)
    cap = ((cnt.max(axis=0) + 127) // 128 * 128).astype(np.int64)
    plan = Plan(cfg, cap)

    # stable sort edges by (core, batch, w, t, widx)
    tb = t // cfg.TB
    order = np.lexsort((widx, t, w, tb, core))
    core_s, t_s, w_s, widx_s = core[order], t[order], w[order], widx[order]
    dtl_s, norm_s = dtl[order], norm[order]
    # rank within (core, t, w) group
    gkey = (core_s * cfg.TPS + t_s) * cfg.NWIN + w_s
    change = np.empty(len(gkey), bool)
    change[0] = True
    change[1:] = gkey[1:] != gkey[:-1]
    gstart = np.flatnonzero(change)
    gsize = np.diff(np.append(gstart, len(gkey)))
    rank = np.arange(len(gkey)) - np.repeat(gstart, gsize)
    pos = plan.group_choff[t_s, w_s] * 128 + rank

    idx16 = np.zeros((NC, plan.L), np.int16)
    dstl = np.full((NC, plan.L), -1.0, np.float32)
    nrm = np.zeros((NC, plan.L), np.float32)
    idx16[core_s, pos] = widx_s
    dstl[core_s, pos] = dtl_s
    nrm[core_s, pos] = norm_s

    # wrapped layouts
    idx_w = idx16.reshape(NC, plan.L // 16, 16).transpose(0, 2, 1)   # [NC,16,L/16]
    idx_w = np.ascontiguousarray(np.tile(idx_w, (1, 8, 1)))          # [NC,128,L/16]
    dstl_w = np.ascontiguousarray(dstl.reshape(NC, plan.total_ch, 128).transpose(0, 2, 1))
    nrm_w = np.ascontiguousarray(nrm.reshape(NC, plan.total_ch, 128).transpose(0, 2, 1))

    # x^T shards, bf16, zero-padded to SH columns, wrapped [128, FIN//128, SH]
    # with [p, k, n] = x[n, k*128 + p] (matches the SBUF matmul slicing).
    x = np.asarray(x).astype(np.float32)
    KQ = cfg.FIN // 128
    xT = np.zeros((NC, 128, KQ, cfg.SH), BF)
    for c in range(NC):
        xt = x[c * cfg.SH_RAW:(c + 1) * cfg.SH_RAW].T.astype(BF)  # [FIN, SH_RAW]
        xT[c, :, :, :cfg.SH_RAW] = xt.reshape(KQ, 128, cfg.SH_RAW).transpose(1, 0, 2)

    W1 = np.asarray(W1).astype(np.float32)
    W2 = np.asarray(W2).astype(np.float32)
    w1_w = np.ascontiguousarray(
        W1.reshape(cfg.FIN // 128, 128, cfg.HID).transpose(1, 0, 2).astype(BF))
    w2_w = np.ascontiguousarray(
        W2.reshape(cfg.HID // 128, 128, cfg.FOUT).transpose(1, 0, 2).astype(BF))
    b1r = np.asarray(b1).astype(BF).reshape(1, cfg.HID)
    b2r = np.asarray(b2).astype(BF).reshape(1, cfg.FOUT)
    iota = np.tile(np.arange(128, dtype=np.float32)[None, :], (128, 1))
    ident = np.eye(128, dtype=np.float32).astype(BF)
    ones = np.ones((1, 128), BF)

    in_maps = []
    for c in range(NC):
        in_maps.append({
            "xT": np.ascontiguousarray(xT[c]),
            "w1": w1_w, "w2": w2_w, "b1r": b1r, "b2r": b2r,
            "iota": iota, "ident": ident, "onesv": ones,
            "idx": idx_w[c], "dstl": dstl_w[c], "nrm": nrm_w[c],
        })
    return plan, in_maps


def _build_nc(cfg, plan):
    # 64KB/partition SWDGE descriptor carveout -> 4096-descriptor ring, so
    # two 16-chunk (2048-idx) gathers can be in flight
    nc = bacc.Bacc("TRN2", num_devices=cfg.NCORES,
                   dynamic_dma_scratch_size=65536)
    KQ = cfg.FIN // 128
    HH = cfg.HID // 128
    GW = 128 if cfg.HID > 64 else 128  # g table row width (bf16) -> 256B rows

    xT = nc.dram_tensor("xT", [128, cfg.FIN // 128, cfg.SH], BF16,
                        kind="ExternalInput")
    w1 = nc.dram_tensor("w1", [128, KQ, cfg.HID], BF16, kind="ExternalInput")
    w2 = nc.dram_tensor("w2", [128, HH, cfg.FOUT], BF16, kind="ExternalInput")
    b1r = nc.dram_tensor("b1r", [1, cfg.HID], BF16, kind="ExternalInput")
    b2r = nc.dram_tensor("b2r", [1, cfg.FOUT], BF16, kind="ExternalInput")
    iota = nc.dram_tensor("iota", [128, 128], F32, kind="ExternalInput")
    ident = nc.dram_tensor("ident", [128, 128], BF16, kind="ExternalInput")
    onesv = nc.dram_tensor("onesv", [1, 128], BF16, kind="ExternalInput")
    idx_in = nc.dram_tensor("idx", [128, plan.L // 16], I16, kind="ExternalInput")
    dstl_in = nc.dram_tensor("dstl", [128, plan.total_ch], F32, kind="ExternalInput")
    nrm_in = nc.dram_tensor("nrm", [128, plan.total_ch], F32, kind="ExternalInput")
    out_ext = nc.dram_tensor("out", [cfg.SH, cfg.FOUT], F32, kind="ExternalOutput")

    groups = [list(range(cfg.NCORES))]

    with tile.TileContext(nc) as tc:
        nc.gpsimd.load_library(mlp_lib)
        tc.no_sync_barrier()
        with (
            tc.tile_pool(name="dram", bufs=1, space="DRAM") as dpool,
            tc.tile_pool(name="const", bufs=1) as cpool,
        ):
            hsh = dpool.tile([cfg.SH, cfg.HID], BF16)
            htab = dpool.tile([cfg.PN, cfg.HID], BF16, addr_space="Shared")
            gsh = dpool.tile([cfg.SH, GW], BF16)
            gtab = dpool.tile([cfg.PN, GW], BF16, addr_space="Shared")

            iota_t = cpool.tile([128, 128], F32)
            id_t = cpool.tile([128, 128], BF16)
            ones_t = cpool.tile([1, 128], BF16)
            b1_t = cpool.tile([1, cfg.HID], BF16)
            b2_t = cpool.tile([1, cfg.FOUT], BF16)
            w2_t = cpool.tile([128, HH, cfg.FOUT], BF16)
            nc.sync.dma_start(iota_t[:], iota[:])
            nc.sync.dma_start(id_t[:], ident[:])
            nc.sync.dma_start(ones_t[:], onesv[:])
            nc.sync.dma_start(b1_t[:], b1r[:])
            nc.sync.dma_start(b2_t[:], b2r[:])
            nc.sync.dma_start(w2_t[:], w2[:])

            # ---------------- phase 1: h = x @ W1 (shard) ----------------
            with (
                tc.tile_pool(name="p1sb", bufs=1) as p1sb,
                tc.tile_pool(name="p1st", bufs=3) as p1st,
                tc.tile_pool(name="p1ps", bufs=2, space="PSUM") as p1ps,
            ):
                xT_t = p1sb.tile([128, KQ, cfg.SH], BF16)
                w1_t = p1sb.tile([128, KQ, cfg.HID], BF16)
                nc.sync.dma_start(xT_t[:], xT[:])
                nc.sync.dma_start(w1_t[:], w1[:])
                for j in range(cfg.TPS):
                    ph = p1ps.tile([128, cfg.HID], F32, tag="ph")
                    for k in range(KQ):
                        nc.tensor.matmul(
                            ph[:], xT_t[:, k, j * 128:(j + 1) * 128], w1_t[:, k, :],
                            start=(k == 0), stop=(k == KQ - 1))
                    hst = p1st.tile([128, cfg.HID], BF16, tag="hst")
                    nc.scalar.activation(hst[:], ph[:],
                                         mybir.ActivationFunctionType.Copy)
                    nc.sync.dma_start(hsh[j * 128:(j + 1) * 128, :], hst[:])

            # ---------------- phase 2: allgather h ----------------
            nc.gpsimd.collective_compute(
                "AllGather", mybir.AluOpType.bypass, replica_groups=groups,
                ins=[hsh[:].opt()], outs=[htab[:].opt()])

            # num_idxs register constants for gathers (Pool regs are scarce:
            # to_reg doesn't dedupe, so cache handles per distinct value)
            nidx_regs = {}

            def nreg(n):
                if n not in nidx_regs:
                    nidx_regs[n] = nc.gpsimd.to_reg(n)
                return nidx_regs[n]

            # ---------------- phases 3+5 shared aggregation loop ----------
            def agg_pass(tab, elem, out_emit, psum_pool, chunk_emit):
                with (
                    tc.tile_pool(name="meta", bufs=2) as mpool,
                    tc.tile_pool(name="gat", bufs=3) as gpool,
                    tc.tile_pool(name="st", bufs=4) as spool,
                ):
                    for b in range(cfg.NB):
                        nch = plan.batch_nch[b]
                        coff = plan.batch_choff[b]
                        idx_b = mpool.tile([128, plan.max_batch_ch * 8], I16, tag="idx")
                        dstl_b = mpool.tile([128, plan.max_batch_ch], F32, tag="dstl")
                        nrm_b = mpool.tile([128, plan.max_batch_ch], F32, tag="nrm")
                        nc.sync.dma_start(idx_b[:, :nch * 8],
                                          idx_in[:, coff * 8:(coff + nch) * 8])
                        nc.sync.dma_start(dstl_b[:, :nch], dstl_in[:, coff:coff + nch])
                        nc.sync.dma_start(nrm_b[:, :nch], nrm_in[:, coff:coff + nch])

                        aggs = {}
                        ctiles = plan.batch_chunk_tiles[b]
                        first, last = plan.batch_first[b], plan.batch_last[b]
                        for (w, c0, c1) in plan.batch_slabs[b]:
                            sc = c1 - c0
                            m_t = gpool.tile([128, cfg.SLAB_CH, elem], BF16, tag="m")
                            nc.gpsimd.dma_gather(
                                m_t[:, :sc, :],
                                tab[w * cfg.WIN:(w + 1) * cfg.WIN, :],
                                idx_b[:, c0 * 8:c1 * 8],
                                sc * 128, nreg(sc * 128), elem)
                            for ci in range(c0, c1):
                                t = ctiles[ci]
                                if t not in aggs:
                                    aggs[t] = psum_pool.tile(
                                        [128, chunk_emit.width], F32,
                                        tag="agg", name=f"agg_b{b}_t{t}")
                                st = spool.tile([128, 128], BF16, tag="st")
                                nc.vector.tensor_scalar(
                                    st[:], iota_t[:],
                                    dstl_b[:, ci:ci + 1], nrm_b[:, ci:ci + 1],
                                    mybir.AluOpType.is_equal, mybir.AluOpType.mult)
                                chunk_emit(aggs[t], st, m_t[:, ci - c0, :],
                                           ci == first[t], ci == last[t])
                        for t in plan.batches[b]:
                            out_emit(t, aggs[t])

            # phase 3 chunk: agg[128d, HID] += S_T.T @ h_rows
            def p3_chunk(agg, st, mrow, is_first, is_last):
                nc.tensor.matmul(agg[:], st[:], mrow, start=is_first, stop=False)
            p3_chunk.width = cfg.HID

            with (
                tc.tile_pool(name="p3ps", bufs=2 * cfg.TB, space="PSUM") as p3ps,
                tc.tile_pool(name="ep", bufs=2) as ep,
                tc.tile_pool(name="eptp", bufs=1, space="PSUM") as eptp,
                tc.tile_pool(name="epg", bufs=1, space="PSUM") as epg,
            ):
                def p3_out(t, agg):
                    # + b1 broadcast, closes the accumulation group
                    nc.tensor.matmul(agg[:], ones_t[:], b1_t[:],
                                     start=False, stop=True)
                    hr = ep.tile([128, cfg.HID], BF16, tag="hr")
                    nc.scalar.activation(hr[:], agg[:],
                                         mybir.ActivationFunctionType.Relu)
                    gp = epg.tile([128, cfg.FOUT], F32, tag="gp")
                    for hh in range(HH):
                        tp = eptp.tile([128, 128], BF16, tag="tp")
                        nc.tensor.transpose(tp[:], hr[:, hh * 128:(hh + 1) * 128],
                                            id_t[:])
                        hrT = ep.tile([128, 128], BF16, tag="hrT")
                        nc.vector.tensor_copy(hrT[:], tp[:])
                        nc.tensor.matmul(gp[:], hrT[:], w2_t[:, hh, :],
                                         start=(hh == 0), stop=(hh == HH - 1))
                    gst = ep.tile([128, GW], BF16, tag="gst")
                    nc.vector.memset(gst[:, cfg.FOUT:], 0.0)
                    nc.scalar.activation(gst[:, :cfg.FOUT], gp[:],
                                         mybir.ActivationFunctionType.Copy)
                    nc.sync.dma_start(gsh[t * 128:(t + 1) * 128, :], gst[:])

                agg_pass(htab, cfg.HID, p3_out, p3ps, p3_chunk)

            # ---------------- phase 4: allgather g ----------------
            nc.gpsimd.collective_compute(
                "AllGather", mybir.AluOpType.bypass, replica_groups=groups,
                ins=[gsh[:].opt()], outs=[gtab[:].opt()])

            # ---------------- phase 5 ----------------
            def p5_chunk(agg, st, mrow, is_first, is_last):
                nc.tensor.matmul(agg[:], st[:], mrow[:, :cfg.FOUT],
                                 start=is_first, stop=False)
            p5_chunk.width = cfg.FOUT

            with (
                tc.tile_pool(name="p5ps", bufs=2 * cfg.TB, space="PSUM") as p5ps,
                tc.tile_pool(name="op", bufs=3) as opool,
            ):
                def p5_out(t, agg):
                    nc.tensor.matmul(agg[:], ones_t[:], b2_t[:],
                                     start=False, stop=True)
                    ost = opool.tile([128, cfg.FOUT], F32, tag="ost")
                    nc.vector.tensor_copy(ost[:], agg[:])
                    nc.sync.dma_start(out_ext[t * 128:(t + 1) * 128, :], ost[:])

                agg_pass(gtab, GW, p5_out, p5ps, p5_chunk)

    nc.compile()
    return nc


def _run(cfg, x, edge_index, edge_weight, W1, b1, W2, b2, **run_kw):
    plan, in_maps = _preprocess(cfg, x, edge_index, edge_weight, W1, b1, W2, b2)
    nc = _build_nc(cfg, plan)
    res = run_bass_kernel_spmd(nc, in_maps, core_ids=list(range(cfg.NCORES)),
                               **run_kw)
    shards = [res.results[c]["out"][:cfg.SH_RAW] for c in range(cfg.NCORES)]
    out = np.concatenate(shards, axis=0).astype(np.float32)
    return out, res


def kernel(x, edge_index, edge_weight, W1, b1, W2, b2):
    cfg = Cfg()
    out, _ = _run(cfg, x, edge_index, edge_weight, W1, b1, W2, b2)
    return out


# revision 22
# speedup vs baseline: 1.7015x; 1.7015x over previous
"""Two-layer GCN (PyG GCNConv semantics) on 8 Trainium2 NeuronCores.

Strategy (graph/data parallel, dst-sharded):
  - Nodes are sharded row-wise across the 8 cores (12500 each, padded to
    12544 = 98*128). Weights are replicated.
  - Phase 1: each core computes its shard of h = x @ W1 (bf16 matmul).
  - Phase 2: AllGather h shards -> full bf16 node-feature table per core.
  - Phase 3: per-core aggregation over incoming edges of its dst shard:
    dma_gather of h[src] rows + one-hot scaled selection matmuls that
    scatter-accumulate into PSUM per 128-dst tile; + b1, ReLU,
    then hr @ W2 (via PE transpose) -> per-node 16-wide g vectors.
  - Phase 4: AllGather g shards -> full g table.
  - Phase 5: same aggregation structure over g rows + b2 -> output shard.

Edge preprocessing (host, numpy): symmetric-norm coefficients, self loops,
edges sorted/grouped per (core, tile-batch, src-window, tile) with
capacities fixed to the max across cores so all 8 cores run one SPMD NEFF.
Index tables are int16 (hardware gather constraint) relative to 25088-row
windows of the padded node table.
"""

import math

import ml_dtypes
import numpy as np

import concourse.bacc as bacc
import concourse.bass as bass
import concourse.mybir as mybir
import concourse.tile as tile
from concourse.bass_utils import run_bass_kernel_spmd
from concourse.library_config import mlp as mlp_lib

F32 = mybir.dt.float32
BF16 = mybir.dt.bfloat16
I16 = mybir.dt.int16
BF = ml_dtypes.bfloat16


class Cfg:
    def __init__(self, N=100000, E=3200000, FIN=512, HID=256, FOUT=16,
                 NCORES=8, TB=3, SLAB_CH=8, MAX_WIN=32768):
        self.N, self.E, self.FIN, self.HID, self.FOUT = N, E, FIN, HID, FOUT
        self.NCORES, self.TB, self.SLAB_CH = NCORES, TB, SLAB_CH
        assert N % NCORES == 0
        self.SH_RAW = N // NCORES
        self.TPS = (self.SH_RAW + 127) // 128          # tiles per shard
        self.SH = self.TPS * 128                        # padded shard rows
        self.PN = NCORES * self.SH                      # padded table rows
        self.W_SH = max(1, min(MAX_WIN // self.SH, NCORES))
        while NCORES % self.W_SH:
            self.W_SH -= 1
        self.WIN = self.W_SH * self.SH                  # window rows (int16-addressable)
        assert self.WIN <= 32768
        self.NWIN = NCORES // self.W_SH
        self.NB = (self.TPS + TB - 1) // TB             # tile batches
        assert FIN % 128 == 0 and HID % 128 == 0 and FOUT <= 128


class Plan:
    """Static (core-independent) edge-stream structure."""

    def __init__(self, cfg, cap):
        # cap: [TPS, NWIN] slot capacity (multiples of 128)
        self.cfg = cfg
        self.cap = cap
        self.batches = [list(range(b * cfg.TB, min((b + 1) * cfg.TB, cfg.TPS)))
                        for b in range(cfg.NB)]
        # per (t, w) chunk offset in the global stream (in chunks of 128 slots)
        self.group_choff = np.zeros((cfg.TPS, cfg.NWIN), np.int64)
        self.batch_choff = []           # chunk offset of each batch
        self.batch_nch = []             # chunks in each batch
        self.batch_slabs = []           # [(w, c0, c1)] chunk ranges (batch-rel)
        self.batch_chunk_tiles = []     # per-chunk tile id
        self.batch_first = []           # tile -> first batch-rel chunk
        self.batch_last = []            # tile -> last batch-rel chunk
        off = 0
        for b, tiles in enumerate(self.batches):
            self.batch_choff.append(off)
            ctiles = []
            slabs = []
            first, last = {}, {}
            for w in range(cfg.NWIN):
                w0 = len(ctiles)
                for t in tiles:
                    self.group_choff[t, w] = off + len(ctiles)
                    nch = cap[t, w] // 128
                    for _ in range(nch):
                        first.setdefault(t, len(ctiles))
                        last[t] = len(ctiles)
                        ctiles.append(t)
                # split into pieces of at most SLAB_CH chunks (each gather's
                # descriptor count must fit the SWDGE ring with headroom;
                # <= SLAB_CH distinct sizes keeps num_idxs register use low)
                c0 = w0
                rem = len(ctiles) - w0
                while rem:
                    p = min(cfg.SLAB_CH, rem)
                    slabs.append((w, c0, c0 + p))
                    c0 += p
                    rem -= p
            self.batch_chunk_tiles.append(ctiles)
            self.batch_slabs.append(slabs)
            self.batch_first.append(first)
            self.batch_last.append(last)
            self.batch_nch.append(len(ctiles))
            off += len(ctiles)
        self.total_ch = off
        self.L = off * 128
        self.max_batch_ch = max(self.batch_nch)


def _preprocess(cfg, x, edge_index, edge_weight, W1, b1, W2, b2):
    N, NC = cfg.N, cfg.NCORES
    src = np.asarray(edge_index[0]).astype(np.int64)
    dst = np.asarray(edge_index[1]).astype(np.int64)
    ew = np.asarray(edge_weight).astype(np.float32)

    # self loops (weight 1.0), symmetric normalization at dst
    deg = np.bincount(dst, weights=ew.astype(np.float64), minlength=N) + 1.0
    dinv = (1.0 / np.sqrt(deg)).astype(np.float32)
    src_f = np.concatenate([src, np.arange(N, dtype=np.int64)])
    dst_f = np.concatenate([dst, np.arange(N, dtype=np.int64)])
    ew_f = np.concatenate([ew, np.ones(N, np.float32)])
    norm = dinv[src_f] * ew_f * dinv[dst_f]

    core = dst_f // cfg.SH_RAW
    dl = dst_f % cfg.SH_RAW
    t = dl // 128
    dtl = (dl % 128).astype(np.float32)
    rsrc = (src_f // cfg.SH_RAW) * cfg.SH + (src_f % cfg.SH_RAW)
    w = rsrc // cfg.WIN
    widx = (rsrc % cfg.WIN).astype(np.int16)

    cnt = np.bincount((core * cfg.TPS + t) * cfg.NWIN + w,
                      minlength=NC * cfg.TPS * cfg.NWIN
                      ).reshape(NC, cfg.TPS, cfg.NWIN)
    cap = ((cnt.max(axis=0) + 127) // 128 * 128).astype(np.int64)
    plan = Plan(cfg, cap)

    # stable sort edges by (core, batch, w, t, widx)
    tb = t // cfg.TB
    order = np.lexsort((widx, t, w, tb, core))
    core_s, t_s, w_s, widx_s = core[order], t[order], w[order], widx[order]
    dtl_s, norm_s = dtl[order], norm[order]
    # rank within (core, t, w) group
    gkey = (core_s * cfg.TPS + t_s) * cfg.NWIN + w_s
    change = np.empty(len(gkey), bool)
    change[0] = True
    change[1:] = gkey[1:] != gkey[:-1]
    gstart = np.flatnonzero(change)
    gsize = np.diff(np.append(gstart, len(gkey)))
    rank = np.arange(len(gkey)) - np.repeat(gstart, gsize)
    pos = plan.group_choff[t_s, w_s] * 128 + rank

    idx16 = np.zeros((NC, plan.L), np.int16)
    dstl = np.full((NC, plan.L), -1.0, np.float32)
    nrm = np.zeros((NC, plan.L), np.float32)
    idx16[core_s, pos] = widx_s
    dstl[core_s, pos] = dtl_s
    nrm[core_s, pos] = norm_s

    # wrapped layouts
    idx_w = idx16.reshape(NC, plan.L // 16, 16).transpose(0, 2, 1)   # [NC,16,L/16]
    idx_w = np.ascontiguousarray(np.tile(idx_w, (1, 8, 1)))          # [NC,128,L/16]
    dstl_w = np.ascontiguousarray(
        dstl.reshape(NC, plan.total_ch, 128).transpose(0, 2, 1).astype(BF))
    nrm_w = np.ascontiguousarray(
        nrm.reshape(NC, plan.total_ch, 128).transpose(0, 2, 1).astype(BF))

    # x^T shards, bf16, zero-padded to SH columns, wrapped [128, FIN//128, SH]
    # with [p, k, n] = x[n, k*128 + p] (matches the SBUF matmul slicing).
    x = np.asarray(x).astype(np.float32)
    KQ = cfg.FIN // 128
    xT = np.zeros((NC, 128, KQ, cfg.SH), BF)
    for c in range(NC):
        xt = x[c * cfg.SH_RAW:(c + 1) * cfg.SH_RAW].T.astype(BF)  # [FIN, SH_RAW]
        xT[c, :, :, :cfg.SH_RAW] = xt.reshape(KQ, 128, cfg.SH_RAW).transpose(1, 0, 2)

    W1 = np.asarray(W1).astype(np.float32)
    W2 = np.asarray(W2).astype(np.float32)
    w1_w = np.ascontiguousarray(
        W1.reshape(cfg.FIN // 128, 128, cfg.HID).transpose(1, 0, 2).astype(BF))
    w2_w = np.ascontiguousarray(
        W2.reshape(cfg.HID // 128, 128, cfg.FOUT).transpose(1, 0, 2).astype(BF))
    b1r = np.asarray(b1).astype(BF).reshape(1, cfg.HID)
    b2r = np.asarray(b2).astype(BF).reshape(1, cfg.FOUT)
    iota = np.tile(np.arange(128, dtype=np.float32)[None, None, :],
                   (128, cfg.SLAB_CH, 1)).astype(BF)
    ident = np.eye(128, dtype=np.float32).astype(BF)
    ones = np.ones((1, 128), BF)

    in_maps = []
    for c in range(NC):
        in_maps.append({
            "xT": np.ascontiguousarray(xT[c]),
            "w1": w1_w, "w2": w2_w, "b1r": b1r, "b2r": b2r,
            "iota": iota, "ident": ident, "onesv": ones,
            "idx": idx_w[c], "dstl": dstl_w[c], "nrm": nrm_w[c],
        })
    return plan, in_maps


def _build_nc(cfg, plan):
    # 64KB/partition SWDGE descriptor carveout; gathers rotate over all 4
    # SWDGE queues (measured 2.6x faster than a single queue)
    nc = bacc.Bacc("TRN2", num_devices=cfg.NCORES,
                   dynamic_dma_scratch_size=65536, num_swdge_queues=4)
    KQ = cfg.FIN // 128
    HH = cfg.HID // 128
    GW = 128 if cfg.HID > 64 else 128  # g table row width (bf16) -> 256B rows

    xT = nc.dram_tensor("xT", [128, cfg.FIN // 128, cfg.SH], BF16,
                        kind="ExternalInput")
    SC = cfg.SLAB_CH
    w1 = nc.dram_tensor("w1", [128, KQ, cfg.HID], BF16, kind="ExternalInput")
    w2 = nc.dram_tensor("w2", [128, HH, cfg.FOUT], BF16, kind="ExternalInput")
    b1r = nc.dram_tensor("b1r", [1, cfg.HID], BF16, kind="ExternalInput")
    b2r = nc.dram_tensor("b2r", [1, cfg.FOUT], BF16, kind="ExternalInput")
    iota = nc.dram_tensor("iota", [128, SC, 128], BF16, kind="ExternalInput")
    ident = nc.dram_tensor("ident", [128, 128], BF16, kind="ExternalInput")
    onesv = nc.dram_tensor("onesv", [1, 128], BF16, kind="ExternalInput")
    idx_in = nc.dram_tensor("idx", [128, plan.L // 16], I16, kind="ExternalInput")
    dstl_in = nc.dram_tensor("dstl", [128, plan.total_ch], BF16, kind="ExternalInput")
    nrm_in = nc.dram_tensor("nrm", [128, plan.total_ch], BF16, kind="ExternalInput")
    out_ext = nc.dram_tensor("out", [cfg.SH, cfg.FOUT], F32, kind="ExternalOutput")

    groups = [list(range(cfg.NCORES))]

    with tile.TileContext(nc) as tc:
        nc.gpsimd.load_library(mlp_lib)
        tc.no_sync_barrier()
        with (
            tc.tile_pool(name="dram", bufs=1, space="DRAM") as dpool,
            tc.tile_pool(name="const", bufs=1) as cpool,
        ):
            hsh = dpool.tile([cfg.SH, cfg.HID], BF16)
            htab = dpool.tile([cfg.PN, cfg.HID], BF16, addr_space="Shared")
            gsh = dpool.tile([cfg.SH, GW], BF16)
            gtab = dpool.tile([cfg.PN, GW], BF16, addr_space="Shared")

            iota_t = cpool.tile([128, SC, 128], BF16)
            id_t = cpool.tile([128, 128], BF16)
            ones_t = cpool.tile([1, 128], BF16)
            b1_t = cpool.tile([1, cfg.HID], BF16)
            b2_t = cpool.tile([1, cfg.FOUT], BF16)
            w2_t = cpool.tile([128, HH, cfg.FOUT], BF16)
            nc.sync.dma_start(iota_t[:], iota[:])
            nc.sync.dma_start(id_t[:], ident[:])
            nc.sync.dma_start(ones_t[:], onesv[:])
            nc.sync.dma_start(b1_t[:], b1r[:])
            nc.sync.dma_start(b2_t[:], b2r[:])
            nc.sync.dma_start(w2_t[:], w2[:])

            # ---------------- phase 1: h = x @ W1 (shard) ----------------
            with (
                tc.tile_pool(name="p1sb", bufs=1) as p1sb,
                tc.tile_pool(name="p1st", bufs=3) as p1st,
                tc.tile_pool(name="p1ps", bufs=2, space="PSUM") as p1ps,
            ):
                xT_t = p1sb.tile([128, KQ, cfg.SH], BF16)
                w1_t = p1sb.tile([128, KQ, cfg.HID], BF16)
                nc.sync.dma_start(xT_t[:], xT[:])
                nc.sync.dma_start(w1_t[:], w1[:])
                for j in range(cfg.TPS):
                    ph = p1ps.tile([128, cfg.HID], F32, tag="ph")
                    for k in range(KQ):
                        nc.tensor.matmul(
                            ph[:], xT_t[:, k, j * 128:(j + 1) * 128], w1_t[:, k, :],
                            start=(k == 0), stop=(k == KQ - 1))
                    hst = p1st.tile([128, cfg.HID], BF16, tag="hst")
                    nc.scalar.activation(hst[:], ph[:],
                                         mybir.ActivationFunctionType.Copy)
                    nc.sync.dma_start(hsh[j * 128:(j + 1) * 128, :], hst[:])

            # ---------------- phase 2: allgather h ----------------
            nc.gpsimd.collective_compute(
                "AllGather", mybir.AluOpType.bypass, replica_groups=groups,
                ins=[hsh[:].opt()], outs=[htab[:].opt()])

            # num_idxs register constants for gathers (Pool regs are scarce:
            # to_reg doesn't dedupe, so cache handles per distinct value)
            nidx_regs = {}

            def nreg(n):
                if n not in nidx_regs:
                    nidx_regs[n] = nc.gpsimd.to_reg(n)
                return nidx_regs[n]

            # ---------------- phases 3+5 shared aggregation loop ----------
            def agg_pass(tab, elem, out_emit, psum_pool, chunk_emit):
                qn = [0]
                with (
                    tc.tile_pool(name="meta", bufs=2) as mpool,
                    tc.tile_pool(name="gat", bufs=6) as gpool,
                    tc.tile_pool(name="st", bufs=3) as spool,
                ):
                    for b in range(cfg.NB):
                        nch = plan.batch_nch[b]
                        coff = plan.batch_choff[b]
                        idx_b = mpool.tile([128, plan.max_batch_ch * 8], I16, tag="idx")
                        dstl_b = mpool.tile([128, plan.max_batch_ch], BF16, tag="dstl")
                        nrm_b = mpool.tile([128, plan.max_batch_ch], BF16, tag="nrm")
                        nc.sync.dma_start(idx_b[:, :nch * 8],
                                          idx_in[:, coff * 8:(coff + nch) * 8])
                        nc.sync.dma_start(dstl_b[:, :nch], dstl_in[:, coff:coff + nch])
                        nc.sync.dma_start(nrm_b[:, :nch], nrm_in[:, coff:coff + nch])

                        aggs = {}
                        ctiles = plan.batch_chunk_tiles[b]
                        first, last = plan.batch_first[b], plan.batch_last[b]
                        for (w, c0, c1) in plan.batch_slabs[b]:
                            sc = c1 - c0
                            m_t = gpool.tile([128, cfg.SLAB_CH, elem], BF16, tag="m")
                            nc.gpsimd.dma_gather(
                                m_t[:, :sc, :],
                                tab[w * cfg.WIN:(w + 1) * cfg.WIN, :],
                                idx_b[:, c0 * 8:c1 * 8],
                                sc * 128, nreg(sc * 128), elem,
                                queue_num=qn[0])
                            qn[0] = (qn[0] + 1) % 2
                            # one-hot selection matrices for the whole slab in
                            # two wide DVE ops (per-chunk tensor_scalar is
                            # ~10x slower than tensor_tensor on HW)
                            eq_t = spool.tile([128, cfg.SLAB_CH, 128], BF16,
                                              tag="eq")
                            st_t = spool.tile([128, cfg.SLAB_CH, 128], BF16,
                                              tag="st")
                            dl_b = dstl_b[:, c0:c1, None].broadcast_to(
                                (128, sc, 128))
                            nr_b = nrm_b[:, c0:c1, None].broadcast_to(
                                (128, sc, 128))
                            nc.vector.tensor_tensor(
                                eq_t[:, :sc, :], iota_t[:, :sc, :], dl_b,
                                mybir.AluOpType.is_equal)
                            nc.vector.tensor_tensor(
                                st_t[:, :sc, :], eq_t[:, :sc, :], nr_b,
                                mybir.AluOpType.mult)
                            for ci in range(c0, c1):
                                t = ctiles[ci]
                                if t not in aggs:
                                    aggs[t] = psum_pool.tile(
                                        [128, chunk_emit.width], F32,
                                        tag="agg", name=f"agg_b{b}_t{t}")
                                chunk_emit(aggs[t], st_t[:, ci - c0, :],
                                           m_t[:, ci - c0, :],
                                           ci == first[t], ci == last[t])
                        for t in plan.batches[b]:
                            out_emit(t, aggs[t])

            # phase 3 chunk: agg[128d, HID] += S_T.T @ h_rows
            def p3_chunk(agg, st, mrow, is_first, is_last):
                nc.tensor.matmul(agg[:], st[:], mrow, start=is_first, stop=False)
            p3_chunk.width = cfg.HID

            with (
                tc.tile_pool(name="p3ps", bufs=2 * cfg.TB, space="PSUM") as p3ps,
                tc.tile_pool(name="ep", bufs=2) as ep,
                tc.tile_pool(name="eptp", bufs=1, space="PSUM") as eptp,
                tc.tile_pool(name="epg", bufs=1, space="PSUM") as epg,
            ):
                def p3_out(t, agg):
                    # + b1 broadcast, closes the accumulation group
                    nc.tensor.matmul(agg[:], ones_t[:], b1_t[:],
                                     start=False, stop=True)
                    hr = ep.tile([128, cfg.HID], BF16, tag="hr")
                    nc.scalar.activation(hr[:], agg[:],
                                         mybir.ActivationFunctionType.Relu)
                    gp = epg.tile([128, cfg.FOUT], F32, tag="gp")
                    for hh in range(HH):
                        tp = eptp.tile([128, 128], BF16, tag="tp")
                        nc.tensor.transpose(tp[:], hr[:, hh * 128:(hh + 1) * 128],
                                            id_t[:])
                        hrT = ep.tile([128, 128], BF16, tag="hrT")
                        nc.vector.tensor_copy(hrT[:], tp[:])
                        nc.tensor.matmul(gp[:], hrT[:], w2_t[:, hh, :],
                                         start=(hh == 0), stop=(hh == HH - 1))
                    gst = ep.tile([128, GW], BF16, tag="gst")
                    nc.vector.memset(gst[:, cfg.FOUT:], 0.0)
                    nc.scalar.activation(gst[:, :cfg.FOUT], gp[:],
                                         mybir.ActivationFunctionType.Copy)
                    nc.sync.dma_start(gsh[t * 128:(t + 1) * 128, :], gst[:])

                agg_pass(htab, cfg.HID, p3_out, p3ps, p3_chunk)

            # ---------------- phase 4: allgather g ----------------
            nc.gpsimd.collective_compute(
                "AllGather", mybir.AluOpType.bypass, replica_groups=groups,
                ins=[gsh[:].opt()], outs=[gtab[:].opt()])

            # ---------------- phase 5 ----------------
            def p5_chunk(agg, st, mrow, is_first, is_last):
                nc.tensor.matmul(agg[:], st[:], mrow[:, :cfg.FOUT],
                                 start=is_first, stop=False)
            p5_chunk.width = cfg.FOUT

            with (
                tc.tile_pool(name="p5ps", bufs=2 * cfg.TB, space="PSUM") as p5ps,
                tc.tile_pool(name="op", bufs=3) as opool,
            ):
                def p5_out(t, agg):
                    nc.tensor.matmul(agg[:], ones_t[:], b2_t[:],
                                     start=False, stop=True)
                    ost = opool.tile([128, cfg.FOUT], F32, tag="ost")
                    nc.vector.tensor_copy(ost[:], agg[:])
                    nc.sync.dma_start(out_ext[t * 128:(t + 1) * 128, :], ost[:])

                agg_pass(gtab, GW, p5_out, p5ps, p5_chunk)

    nc.compile()
    return nc


def _run(cfg, x, edge_index, edge_weight, W1, b1, W2, b2, **run_kw):
    plan, in_maps = _preprocess(cfg, x, edge_index, edge_weight, W1, b1, W2, b2)
    nc = _build_nc(cfg, plan)
    res = run_bass_kernel_spmd(nc, in_maps, core_ids=list(range(cfg.NCORES)),
                               **run_kw)
    shards = [res.results[c]["out"][:cfg.SH_RAW] for c in range(cfg.NCORES)]
    out = np.concatenate(shards, axis=0).astype(np.float32)
    return out, res


def kernel(x, edge_index, edge_weight, W1, b1, W2, b2):
    cfg = Cfg()
    out, _ = _run(cfg, x, edge_index, edge_weight, W1, b1, W2, b2)
    return out


# revision 23
# speedup vs baseline: 1.7152x; 1.0081x over previous
"""Two-layer GCN (PyG GCNConv semantics) on 8 Trainium2 NeuronCores.

Strategy (graph/data parallel, dst-sharded):
  - Nodes are sharded row-wise across the 8 cores (12500 each, padded to
    12544 = 98*128). Weights are replicated.
  - Phase 1: each core computes its shard of h = x @ W1 (bf16 matmul).
  - Phase 2: AllGather h shards -> full bf16 node-feature table per core.
  - Phase 3: per-core aggregation over incoming edges of its dst shard:
    dma_gather of h[src] rows + one-hot scaled selection matmuls that
    scatter-accumulate into PSUM per 128-dst tile; + b1, ReLU,
    then hr @ W2 (via PE transpose) -> per-node 16-wide g vectors.
  - Phase 4: AllGather g shards -> full g table.
  - Phase 5: same aggregation structure over g rows + b2 -> output shard.

Edge preprocessing (host, numpy): symmetric-norm coefficients, self loops,
edges sorted/grouped per (core, tile-batch, src-window, tile) with
capacities fixed to the max across cores so all 8 cores run one SPMD NEFF.
Index tables are int16 (hardware gather constraint) relative to 25088-row
windows of the padded node table.
"""

import math

import ml_dtypes
import numpy as np

import concourse.bacc as bacc
import concourse.bass as bass
import concourse.mybir as mybir
import concourse.tile as tile
from concourse.bass_utils import run_bass_kernel_spmd
from concourse.library_config import mlp as mlp_lib

F32 = mybir.dt.float32
BF16 = mybir.dt.bfloat16
I16 = mybir.dt.int16
BF = ml_dtypes.bfloat16


class Cfg:
    def __init__(self, N=100000, E=3200000, FIN=512, HID=256, FOUT=16,
                 NCORES=8, TB=3, SLAB_CH=8, MAX_WIN=32768):
        self.N, self.E, self.FIN, self.HID, self.FOUT = N, E, FIN, HID, FOUT
        self.NCORES, self.TB, self.SLAB_CH = NCORES, TB, SLAB_CH
        assert N % NCORES == 0
        self.SH_RAW = N // NCORES
        self.TPS = (self.SH_RAW + 127) // 128          # tiles per shard
        self.SH = self.TPS * 128                        # padded shard rows
        self.PN = NCORES * self.SH                      # padded table rows
        self.W_SH = max(1, min(MAX_WIN // self.SH, NCORES))
        while NCORES % self.W_SH:
            self.W_SH -= 1
        self.WIN = self.W_SH * self.SH                  # window rows (int16-addressable)
        assert self.WIN <= 32768
        self.NWIN = NCORES // self.W_SH
        self.NB = (self.TPS + TB - 1) // TB             # tile batches
        assert FIN % 128 == 0 and HID % 128 == 0 and FOUT <= 128


class Plan:
    """Static (core-independent) edge-stream structure."""

    def __init__(self, cfg, cap):
        # cap: [TPS, NWIN] slot capacity (multiples of 128)
        self.cfg = cfg
        self.cap = cap
        self.batches = [list(range(b * cfg.TB, min((b + 1) * cfg.TB, cfg.TPS)))
                        for b in range(cfg.NB)]
        # per (t, w) chunk offset in the global stream (in chunks of 128 slots)
        self.group_choff = np.zeros((cfg.TPS, cfg.NWIN), np.int64)
        self.batch_choff = []           # chunk offset of each batch
        self.batch_nch = []             # chunks in each batch
        self.batch_slabs = []           # [(w, c0, c1)] chunk ranges (batch-rel)
        self.batch_chunk_tiles = []     # per-chunk tile id
        self.batch_first = []           # tile -> first batch-rel chunk
        self.batch_last = []            # tile -> last batch-rel chunk
        off = 0
        for b, tiles in enumerate(self.batches):
            self.batch_choff.append(off)
            ctiles = []
            slabs = []
            first, last = {}, {}
            for w in range(cfg.NWIN):
                w0 = len(ctiles)
                for t in tiles:
                    self.group_choff[t, w] = off + len(ctiles)
                    nch = cap[t, w] // 128
                    for _ in range(nch):
                        first.setdefault(t, len(ctiles))
                        last[t] = len(ctiles)
                        ctiles.append(t)
                # split into pieces of at most SLAB_CH chunks (each gather's
                # descriptor count must fit the SWDGE ring with headroom;
                # <= SLAB_CH distinct sizes keeps num_idxs register use low)
                c0 = w0
                rem = len(ctiles) - w0
                while rem:
                    p = min(cfg.SLAB_CH, rem)
                    slabs.append((w, c0, c0 + p))
                    c0 += p
                    rem -= p
            self.batch_chunk_tiles.append(ctiles)
            self.batch_slabs.append(slabs)
            self.batch_first.append(first)
            self.batch_last.append(last)
            self.batch_nch.append(len(ctiles))
            off += len(ctiles)
        self.total_ch = off
        self.L = off * 128
        self.max_batch_ch = max(self.batch_nch)


def _preprocess(cfg, x, edge_index, edge_weight, W1, b1, W2, b2):
    N, NC = cfg.N, cfg.NCORES
    src = np.asarray(edge_index[0]).astype(np.int64)
    dst = np.asarray(edge_index[1]).astype(np.int64)
    ew = np.asarray(edge_weight).astype(np.float32)

    # self loops (weight 1.0), symmetric normalization at dst
    deg = np.bincount(dst, weights=ew.astype(np.float64), minlength=N) + 1.0
    dinv = (1.0 / np.sqrt(deg)).astype(np.float32)
    src_f = np.concatenate([src, np.arange(N, dtype=np.int64)])
    dst_f = np.concatenate([dst, np.arange(N, dtype=np.int64)])
    ew_f = np.concatenate([ew, np.ones(N, np.float32)])
    norm = dinv[src_f] * ew_f * dinv[dst_f]

    core = dst_f // cfg.SH_RAW
    dl = dst_f % cfg.SH_RAW
    t = dl // 128
    dtl = (dl % 128).astype(np.float32)
    rsrc = (src_f // cfg.SH_RAW) * cfg.SH + (src_f % cfg.SH_RAW)
    w = rsrc // cfg.WIN
    widx = (rsrc % cfg.WIN).astype(np.int16)

    cnt = np.bincount((core * cfg.TPS + t) * cfg.NWIN + w,
                      minlength=NC * cfg.TPS * cfg.NWIN
                      ).reshape(NC, cfg.TPS, cfg.NWIN)
    cap = ((cnt.max(axis=0) + 127) // 128 * 128).astype(np.int64)
    plan = Plan(cfg, cap)

    # stable sort edges by (core, batch, w, t, widx)
    tb = t // cfg.TB
    order = np.lexsort((widx, t, w, tb, core))
    core_s, t_s, w_s, widx_s = core[order], t[order], w[order], widx[order]
    dtl_s, norm_s = dtl[order], norm[order]
    # rank within (core, t, w) group
    gkey = (core_s * cfg.TPS + t_s) * cfg.NWIN + w_s
    change = np.empty(len(gkey), bool)
    change[0] = True
    change[1:] = gkey[1:] != gkey[:-1]
    gstart = np.flatnonzero(change)
    gsize = np.diff(np.append(gstart, len(gkey)))
    rank = np.arange(len(gkey)) - np.repeat(gstart, gsize)
    pos = plan.group_choff[t_s, w_s] * 128 + rank

    idx16 = np.zeros((NC, plan.L), np.int16)
    dstl = np.full((NC, plan.L), -1.0, np.float32)
    nrm = np.zeros((NC, plan.L), np.float32)
    idx16[core_s, pos] = widx_s
    dstl[core_s, pos] = dtl_s
    nrm[core_s, pos] = norm_s

    # wrapped layouts
    idx_w = idx16.reshape(NC, plan.L // 16, 16).transpose(0, 2, 1)   # [NC,16,L/16]
    idx_w = np.ascontiguousarray(np.tile(idx_w, (1, 8, 1)))          # [NC,128,L/16]
    dstl_w = np.ascontiguousarray(
        dstl.reshape(NC, plan.total_ch, 128).transpose(0, 2, 1).astype(BF))
    nrm_w = np.ascontiguousarray(
        nrm.reshape(NC, plan.total_ch, 128).transpose(0, 2, 1).astype(BF))

    # x^T shards, bf16, zero-padded to SH columns, wrapped [128, FIN//128, SH]
    # with [p, k, n] = x[n, k*128 + p] (matches the SBUF matmul slicing).
    x = np.asarray(x).astype(np.float32)
    KQ = cfg.FIN // 128
    xT = np.zeros((NC, 128, KQ, cfg.SH), BF)
    for c in range(NC):
        xt = x[c * cfg.SH_RAW:(c + 1) * cfg.SH_RAW].T.astype(BF)  # [FIN, SH_RAW]
        xT[c, :, :, :cfg.SH_RAW] = xt.reshape(KQ, 128, cfg.SH_RAW).transpose(1, 0, 2)

    W1 = np.asarray(W1).astype(np.float32)
    W2 = np.asarray(W2).astype(np.float32)
    w1_w = np.ascontiguousarray(
        W1.reshape(cfg.FIN // 128, 128, cfg.HID).transpose(1, 0, 2).astype(BF))
    w2_w = np.ascontiguousarray(
        W2.reshape(cfg.HID // 128, 128, cfg.FOUT).transpose(1, 0, 2).astype(BF))
    b1r = np.asarray(b1).astype(BF).reshape(1, cfg.HID)
    b2r = np.asarray(b2).astype(BF).reshape(1, cfg.FOUT)
    iota = np.tile(np.arange(128, dtype=np.float32)[None, None, :],
                   (128, cfg.SLAB_CH, 1)).astype(BF)
    ident = np.eye(128, dtype=np.float32).astype(BF)
    ones = np.ones((1, 128), BF)

    in_maps = []
    for c in range(NC):
        in_maps.append({
            "xT": np.ascontiguousarray(xT[c]),
            "w1": w1_w, "w2": w2_w, "b1r": b1r, "b2r": b2r,
            "iota": iota, "ident": ident, "onesv": ones,
            "idx": idx_w[c], "dstl": dstl_w[c], "nrm": nrm_w[c],
        })
    return plan, in_maps


def _build_nc(cfg, plan):
    # 64KB/partition SWDGE descriptor carveout; gathers rotate over 2 SWDGE
    # queues (measured ~2x faster than a single queue; 4-queue rotation
    # crashed at full scale once). Declaring only the 2 used queues gives
    # each the largest possible descriptor ring.
    nc = bacc.Bacc("TRN2", num_devices=cfg.NCORES,
                   dynamic_dma_scratch_size=65536, num_swdge_queues=2)
    KQ = cfg.FIN // 128
    HH = cfg.HID // 128
    GW = 128 if cfg.HID > 64 else 128  # g table row width (bf16) -> 256B rows

    xT = nc.dram_tensor("xT", [128, cfg.FIN // 128, cfg.SH], BF16,
                        kind="ExternalInput")
    SC = cfg.SLAB_CH
    w1 = nc.dram_tensor("w1", [128, KQ, cfg.HID], BF16, kind="ExternalInput")
    w2 = nc.dram_tensor("w2", [128, HH, cfg.FOUT], BF16, kind="ExternalInput")
    b1r = nc.dram_tensor("b1r", [1, cfg.HID], BF16, kind="ExternalInput")
    b2r = nc.dram_tensor("b2r", [1, cfg.FOUT], BF16, kind="ExternalInput")
    iota = nc.dram_tensor("iota", [128, SC, 128], BF16, kind="ExternalInput")
    ident = nc.dram_tensor("ident", [128, 128], BF16, kind="ExternalInput")
    onesv = nc.dram_tensor("onesv", [1, 128], BF16, kind="ExternalInput")
    idx_in = nc.dram_tensor("idx", [128, plan.L // 16], I16, kind="ExternalInput")
    dstl_in = nc.dram_tensor("dstl", [128, plan.total_ch], BF16, kind="ExternalInput")
    nrm_in = nc.dram_tensor("nrm", [128, plan.total_ch], BF16, kind="ExternalInput")
    out_ext = nc.dram_tensor("out", [cfg.SH, cfg.FOUT], F32, kind="ExternalOutput")

    groups = [list(range(cfg.NCORES))]

    with tile.TileContext(nc) as tc:
        nc.gpsimd.load_library(mlp_lib)
        tc.no_sync_barrier()
        with (
            tc.tile_pool(name="dram", bufs=1, space="DRAM") as dpool,
            tc.tile_pool(name="const", bufs=1) as cpool,
        ):
            hsh = dpool.tile([cfg.SH, cfg.HID], BF16)
            htab = dpool.tile([cfg.PN, cfg.HID], BF16, addr_space="Shared")
            gsh = dpool.tile([cfg.SH, GW], BF16)
            gtab = dpool.tile([cfg.PN, GW], BF16, addr_space="Shared")

            iota_t = cpool.tile([128, SC, 128], BF16)
            id_t = cpool.tile([128, 128], BF16)
            ones_t = cpool.tile([1, 128], BF16)
            b1_t = cpool.tile([1, cfg.HID], BF16)
            b2_t = cpool.tile([1, cfg.FOUT], BF16)
            w2_t = cpool.tile([128, HH, cfg.FOUT], BF16)
            nc.sync.dma_start(iota_t[:], iota[:])
            nc.sync.dma_start(id_t[:], ident[:])
            nc.sync.dma_start(ones_t[:], onesv[:])
            nc.sync.dma_start(b1_t[:], b1r[:])
            nc.sync.dma_start(b2_t[:], b2r[:])
            nc.sync.dma_start(w2_t[:], w2[:])

            # ---------------- phase 1: h = x @ W1 (shard) ----------------
            with (
                tc.tile_pool(name="p1sb", bufs=1) as p1sb,
                tc.tile_pool(name="p1st", bufs=3) as p1st,
                tc.tile_pool(name="p1ps", bufs=2, space="PSUM") as p1ps,
            ):
                xT_t = p1sb.tile([128, KQ, cfg.SH], BF16)
                w1_t = p1sb.tile([128, KQ, cfg.HID], BF16)
                nc.sync.dma_start(xT_t[:], xT[:])
                nc.sync.dma_start(w1_t[:], w1[:])
                for j in range(cfg.TPS):
                    ph = p1ps.tile([128, cfg.HID], F32, tag="ph")
                    for k in range(KQ):
                        nc.tensor.matmul(
                            ph[:], xT_t[:, k, j * 128:(j + 1) * 128], w1_t[:, k, :],
                            start=(k == 0), stop=(k == KQ - 1))
                    hst = p1st.tile([128, cfg.HID], BF16, tag="hst")
                    nc.scalar.activation(hst[:], ph[:],
                                         mybir.ActivationFunctionType.Copy)
                    nc.sync.dma_start(hsh[j * 128:(j + 1) * 128, :], hst[:])

            # ---------------- phase 2: allgather h ----------------
            nc.gpsimd.collective_compute(
                "AllGather", mybir.AluOpType.bypass, replica_groups=groups,
                ins=[hsh[:].opt()], outs=[htab[:].opt()])

            # num_idxs register constants for gathers (Pool regs are scarce:
            # to_reg doesn't dedupe, so cache handles per distinct value)
            nidx_regs = {}

            def nreg(n):
                if n not in nidx_regs:
                    nidx_regs[n] = nc.gpsimd.to_reg(n)
                return nidx_regs[n]

            # ---------------- phases 3+5 shared aggregation loop ----------
            def agg_pass(tab, elem, out_emit, psum_pool, chunk_emit):
                qn = [0]
                with (
                    tc.tile_pool(name="meta", bufs=2) as mpool,
                    tc.tile_pool(name="gat", bufs=6) as gpool,
                    tc.tile_pool(name="st", bufs=3) as spool,
                ):
                    for b in range(cfg.NB):
                        nch = plan.batch_nch[b]
                        coff = plan.batch_choff[b]
                        idx_b = mpool.tile([128, plan.max_batch_ch * 8], I16, tag="idx")
                        dstl_b = mpool.tile([128, plan.max_batch_ch], BF16, tag="dstl")
                        nrm_b = mpool.tile([128, plan.max_batch_ch], BF16, tag="nrm")
                        nc.sync.dma_start(idx_b[:, :nch * 8],
                                          idx_in[:, coff * 8:(coff + nch) * 8])
                        nc.sync.dma_start(dstl_b[:, :nch], dstl_in[:, coff:coff + nch])
                        nc.sync.dma_start(nrm_b[:, :nch], nrm_in[:, coff:coff + nch])

                        aggs = {}
                        ctiles = plan.batch_chunk_tiles[b]
                        first, last = plan.batch_first[b], plan.batch_last[b]
                        for (w, c0, c1) in plan.batch_slabs[b]:
                            sc = c1 - c0
                            m_t = gpool.tile([128, cfg.SLAB_CH, elem], BF16, tag="m")
                            nc.gpsimd.dma_gather(
                                m_t[:, :sc, :],
                                tab[w * cfg.WIN:(w + 1) * cfg.WIN, :],
                                idx_b[:, c0 * 8:c1 * 8],
                                sc * 128, nreg(sc * 128), elem,
                                queue_num=qn[0])
                            qn[0] = (qn[0] + 1) % 2
                            # one-hot selection matrices for the whole slab in
                            # two wide DVE ops (per-chunk tensor_scalar is
                            # ~10x slower than tensor_tensor on HW)
                            eq_t = spool.tile([128, cfg.SLAB_CH, 128], BF16,
                                              tag="eq")
                            st_t = spool.tile([128, cfg.SLAB_CH, 128], BF16,
                                              tag="st")
                            dl_b = dstl_b[:, c0:c1, None].broadcast_to(
                                (128, sc, 128))
                            nr_b = nrm_b[:, c0:c1, None].broadcast_to(
                                (128, sc, 128))
                            nc.vector.tensor_tensor(
                                eq_t[:, :sc, :], iota_t[:, :sc, :], dl_b,
                                mybir.AluOpType.is_equal)
                            nc.vector.tensor_tensor(
                                st_t[:, :sc, :], eq_t[:, :sc, :], nr_b,
                                mybir.AluOpType.mult)
                            for ci in range(c0, c1):
                                t = ctiles[ci]
                                if t not in aggs:
                                    aggs[t] = psum_pool.tile(
                                        [128, chunk_emit.width], F32,
                                        tag="agg", name=f"agg_b{b}_t{t}")
                                chunk_emit(aggs[t], st_t[:, ci - c0, :],
                                           m_t[:, ci - c0, :],
                                           ci == first[t], ci == last[t])
                        for t in plan.batches[b]:
                            out_emit(t, aggs[t])

            # phase 3 chunk: agg[128d, HID] += S_T.T @ h_rows
            def p3_chunk(agg, st, mrow, is_first, is_last):
                nc.tensor.matmul(agg[:], st[:], mrow, start=is_first, stop=False)
            p3_chunk.width = cfg.HID

            with (
                tc.tile_pool(name="p3ps", bufs=2 * cfg.TB, space="PSUM") as p3ps,
                tc.tile_pool(name="ep", bufs=2) as ep,
                tc.tile_pool(name="eptp", bufs=1, space="PSUM") as eptp,
                tc.tile_pool(name="epg", bufs=1, space="PSUM") as epg,
            ):
                def p3_out(t, agg):
                    # + b1 broadcast, closes the accumulation group
                    nc.tensor.matmul(agg[:], ones_t[:], b1_t[:],
                                     start=False, stop=True)
                    hr = ep.tile([128, cfg.HID], BF16, tag="hr")
                    nc.scalar.activation(hr[:], agg[:],
                                         mybir.ActivationFunctionType.Relu)
                    gp = epg.tile([128, cfg.FOUT], F32, tag="gp")
                    for hh in range(HH):
                        tp = eptp.tile([128, 128], BF16, tag="tp")
                        nc.tensor.transpose(tp[:], hr[:, hh * 128:(hh + 1) * 128],
                                            id_t[:])
                        hrT = ep.tile([128, 128], BF16, tag="hrT")
                        nc.vector.tensor_copy(hrT[:], tp[:])
                        nc.tensor.matmul(gp[:], hrT[:], w2_t[:, hh, :],
                                         start=(hh == 0), stop=(hh == HH - 1))
                    gst = ep.tile([128, GW], BF16, tag="gst")
                    nc.vector.memset(gst[:, cfg.FOUT:], 0.0)
                    nc.scalar.activation(gst[:, :cfg.FOUT], gp[:],
                                         mybir.ActivationFunctionType.Copy)
                    nc.sync.dma_start(gsh[t * 128:(t + 1) * 128, :], gst[:])

                agg_pass(htab, cfg.HID, p3_out, p3ps, p3_chunk)

            # ---------------- phase 4: allgather g ----------------
            nc.gpsimd.collective_compute(
                "AllGather", mybir.AluOpType.bypass, replica_groups=groups,
                ins=[gsh[:].opt()], outs=[gtab[:].opt()])

            # ---------------- phase 5 ----------------
            def p5_chunk(agg, st, mrow, is_first, is_last):
                nc.tensor.matmul(agg[:], st[:], mrow[:, :cfg.FOUT],
                                 start=is_first, stop=False)
            p5_chunk.width = cfg.FOUT

            with (
                tc.tile_pool(name="p5ps", bufs=2 * cfg.TB, space="PSUM") as p5ps,
                tc.tile_pool(name="op", bufs=3) as opool,
            ):
                def p5_out(t, agg):
                    nc.tensor.matmul(agg[:], ones_t[:], b2_t[:],
                                     start=False, stop=True)
                    ost = opool.tile([128, cfg.FOUT], F32, tag="ost")
                    nc.vector.tensor_copy(ost[:], agg[:])
                    nc.sync.dma_start(out_ext[t * 128:(t + 1) * 128, :], ost[:])

                agg_pass(gtab, GW, p5_out, p5ps, p5_chunk)

    nc.compile()
    return nc


def _run(cfg, x, edge_index, edge_weight, W1, b1, W2, b2, **run_kw):
    plan, in_maps = _preprocess(cfg, x, edge_index, edge_weight, W1, b1, W2, b2)
    nc = _build_nc(cfg, plan)
    res = run_bass_kernel_spmd(nc, in_maps, core_ids=list(range(cfg.NCORES)),
                               **run_kw)
    shards = [res.results[c]["out"][:cfg.SH_RAW] for c in range(cfg.NCORES)]
    out = np.concatenate(shards, axis=0).astype(np.float32)
    return out, res


def kernel(x, edge_index, edge_weight, W1, b1, W2, b2):
    cfg = Cfg()
    out, _ = _run(cfg, x, edge_index, edge_weight, W1, b1, W2, b2)
    return out


# revision 24
# speedup vs baseline: 2.1426x; 1.2492x over previous
"""Two-layer GCN (PyG GCNConv semantics) on 8 Trainium2 NeuronCores.

Strategy (graph/data parallel, dst-sharded):
  - Nodes are sharded row-wise across the 8 cores (12500 each, padded to
    12544 = 98*128). Weights are replicated.
  - Phase 1: each core computes its shard of h = x @ W1 (bf16 matmul).
  - Phase 2: AllGather h shards -> full bf16 node-feature table per core.
  - Phase 3: per-core aggregation over incoming edges of its dst shard:
    dma_gather of h[src] rows + one-hot scaled selection matmuls that
    scatter-accumulate into PSUM per 128-dst tile; + b1, ReLU,
    then hr @ W2 (via PE transpose) -> per-node 16-wide g vectors.
  - Phase 4: AllGather g shards -> full g table.
  - Phase 5: same aggregation structure over g rows + b2 -> output shard.

Edge preprocessing (host, numpy): symmetric-norm coefficients, self loops,
edges sorted/grouped per (core, tile-batch, src-window, tile) with
capacities fixed to the max across cores so all 8 cores run one SPMD NEFF.
Index tables are int16 (hardware gather constraint) relative to 25088-row
windows of the padded node table.
"""

import math

import ml_dtypes
import numpy as np

import concourse.bacc as bacc
import concourse.bass as bass
import concourse.mybir as mybir
import concourse.tile as tile
from concourse.bass_utils import run_bass_kernel_spmd
from concourse.library_config import mlp as mlp_lib

F32 = mybir.dt.float32
BF16 = mybir.dt.bfloat16
I16 = mybir.dt.int16
BF = ml_dtypes.bfloat16


class Cfg:
    def __init__(self, N=100000, E=3200000, FIN=512, HID=256, FOUT=16,
                 NCORES=8, TB=3, SLAB_CH=8, MAX_WIN=32768):
        self.N, self.E, self.FIN, self.HID, self.FOUT = N, E, FIN, HID, FOUT
        self.NCORES, self.TB, self.SLAB_CH = NCORES, TB, SLAB_CH
        assert N % NCORES == 0
        self.SH_RAW = N // NCORES
        self.TPS = (self.SH_RAW + 127) // 128          # tiles per shard
        self.SH = self.TPS * 128                        # padded shard rows
        self.PN = NCORES * self.SH                      # padded table rows
        self.W_SH = max(1, min(MAX_WIN // self.SH, NCORES))
        while NCORES % self.W_SH:
            self.W_SH -= 1
        self.WIN = self.W_SH * self.SH                  # window rows (int16-addressable)
        assert self.WIN <= 32768
        self.NWIN = NCORES // self.W_SH
        self.NB = (self.TPS + TB - 1) // TB             # tile batches
        assert FIN % 128 == 0 and HID % 128 == 0 and FOUT <= 128


class Plan:
    """Static (core-independent) edge-stream structure."""

    def __init__(self, cfg, cap):
        # cap: [TPS, NWIN] slot capacity (multiples of 128)
        self.cfg = cfg
        self.cap = cap
        self.batches = [list(range(b * cfg.TB, min((b + 1) * cfg.TB, cfg.TPS)))
                        for b in range(cfg.NB)]
        # per (t, w) chunk offset in the global stream (in chunks of 128 slots)
        self.group_choff = np.zeros((cfg.TPS, cfg.NWIN), np.int64)
        self.batch_choff = []           # chunk offset of each batch
        self.batch_nch = []             # chunks in each batch
        self.batch_slabs = []           # [(w, c0, c1)] chunk ranges (batch-rel)
        self.batch_chunk_tiles = []     # per-chunk tile id
        self.batch_first = []           # tile -> first batch-rel chunk
        self.batch_last = []            # tile -> last batch-rel chunk
        off = 0
        for b, tiles in enumerate(self.batches):
            self.batch_choff.append(off)
            ctiles = []
            slabs = []
            first, last = {}, {}
            for w in range(cfg.NWIN):
                w0 = len(ctiles)
                for t in tiles:
                    self.group_choff[t, w] = off + len(ctiles)
                    nch = cap[t, w] // 128
                    for _ in range(nch):
                        first.setdefault(t, len(ctiles))
                        last[t] = len(ctiles)
                        ctiles.append(t)
                # split into pieces of at most SLAB_CH chunks (each gather's
                # descriptor count must fit the SWDGE ring with headroom;
                # <= SLAB_CH distinct sizes keeps num_idxs register use low)
                c0 = w0
                rem = len(ctiles) - w0
                while rem:
                    p = min(cfg.SLAB_CH, rem)
                    slabs.append((w, c0, c0 + p))
                    c0 += p
                    rem -= p
            self.batch_chunk_tiles.append(ctiles)
            self.batch_slabs.append(slabs)
            self.batch_first.append(first)
            self.batch_last.append(last)
            self.batch_nch.append(len(ctiles))
            off += len(ctiles)
        self.total_ch = off
        self.L = off * 128
        self.max_batch_ch = max(self.batch_nch)


def _preprocess(cfg, x, edge_index, edge_weight, W1, b1, W2, b2):
    N, NC = cfg.N, cfg.NCORES
    src = np.asarray(edge_index[0]).astype(np.int64)
    dst = np.asarray(edge_index[1]).astype(np.int64)
    ew = np.asarray(edge_weight).astype(np.float32)

    # self loops (weight 1.0), symmetric normalization at dst
    deg = np.bincount(dst, weights=ew.astype(np.float64), minlength=N) + 1.0
    dinv = (1.0 / np.sqrt(deg)).astype(np.float32)
    src_f = np.concatenate([src, np.arange(N, dtype=np.int64)])
    dst_f = np.concatenate([dst, np.arange(N, dtype=np.int64)])
    ew_f = np.concatenate([ew, np.ones(N, np.float32)])
    norm = dinv[src_f] * ew_f * dinv[dst_f]

    core = dst_f // cfg.SH_RAW
    dl = dst_f % cfg.SH_RAW
    t = dl // 128
    dtl = (dl % 128).astype(np.float32)
    rsrc = (src_f // cfg.SH_RAW) * cfg.SH + (src_f % cfg.SH_RAW)
    w = rsrc // cfg.WIN
    widx = (rsrc % cfg.WIN).astype(np.int16)

    cnt = np.bincount((core * cfg.TPS + t) * cfg.NWIN + w,
                      minlength=NC * cfg.TPS * cfg.NWIN
                      ).reshape(NC, cfg.TPS, cfg.NWIN)
    cap = ((cnt.max(axis=0) + 127) // 128 * 128).astype(np.int64)
    plan = Plan(cfg, cap)

    # stable sort edges by (core, batch, w, t, widx)
    tb = t // cfg.TB
    order = np.lexsort((widx, t, w, tb, core))
    core_s, t_s, w_s, widx_s = core[order], t[order], w[order], widx[order]
    dtl_s, norm_s = dtl[order], norm[order]
    # rank within (core, t, w) group
    gkey = (core_s * cfg.TPS + t_s) * cfg.NWIN + w_s
    change = np.empty(len(gkey), bool)
    change[0] = True
    change[1:] = gkey[1:] != gkey[:-1]
    gstart = np.flatnonzero(change)
    gsize = np.diff(np.append(gstart, len(gkey)))
    rank = np.arange(len(gkey)) - np.repeat(gstart, gsize)
    pos = plan.group_choff[t_s, w_s] * 128 + rank

    idx16 = np.zeros((NC, plan.L), np.int16)
    dstl = np.full((NC, plan.L), -1.0, np.float32)
    nrm = np.zeros((NC, plan.L), np.float32)
    idx16[core_s, pos] = widx_s
    dstl[core_s, pos] = dtl_s
    nrm[core_s, pos] = norm_s

    # wrapped layouts
    idx_w = idx16.reshape(NC, plan.L // 16, 16).transpose(0, 2, 1)   # [NC,16,L/16]
    idx_w = np.ascontiguousarray(np.tile(idx_w, (1, 8, 1)))          # [NC,128,L/16]
    dstl_w = np.ascontiguousarray(
        dstl.reshape(NC, plan.total_ch, 128).transpose(0, 2, 1).astype(BF))
    nrm_w = np.ascontiguousarray(
        nrm.reshape(NC, plan.total_ch, 128).transpose(0, 2, 1).astype(BF))

    # x^T shards, bf16, zero-padded to SH columns, wrapped [128, FIN//128, SH]
    # with [p, k, n] = x[n, k*128 + p] (matches the SBUF matmul slicing).
    x = np.asarray(x).astype(np.float32)
    KQ = cfg.FIN // 128
    xT = np.zeros((NC, 128, KQ, cfg.SH), BF)
    for c in range(NC):
        xt = x[c * cfg.SH_RAW:(c + 1) * cfg.SH_RAW].T.astype(BF)  # [FIN, SH_RAW]
        xT[c, :, :, :cfg.SH_RAW] = xt.reshape(KQ, 128, cfg.SH_RAW).transpose(1, 0, 2)

    W1 = np.asarray(W1).astype(np.float32)
    W2 = np.asarray(W2).astype(np.float32)
    w1_w = np.ascontiguousarray(
        W1.reshape(cfg.FIN // 128, 128, cfg.HID).transpose(1, 0, 2).astype(BF))
    w2_w = np.ascontiguousarray(
        W2.reshape(cfg.HID // 128, 128, cfg.FOUT).transpose(1, 0, 2).astype(BF))
    b1r = np.asarray(b1).astype(BF).reshape(1, cfg.HID)
    b2r = np.asarray(b2).astype(BF).reshape(1, cfg.FOUT)
    iota = np.tile(np.arange(128, dtype=np.float32)[None, None, :],
                   (128, cfg.SLAB_CH, 1)).astype(BF)
    ident = np.eye(128, dtype=np.float32).astype(BF)
    ones = np.ones((1, 128), BF)

    in_maps = []
    for c in range(NC):
        in_maps.append({
            "xT": np.ascontiguousarray(xT[c]),
            "w1": w1_w, "w2": w2_w, "b1r": b1r, "b2r": b2r,
            "iota": iota, "ident": ident, "onesv": ones,
            "idx": idx_w[c], "dstl": dstl_w[c], "nrm": nrm_w[c],
        })
    return plan, in_maps


def _build_nc(cfg, plan):
    # 64KB/partition SWDGE descriptor carveout; gathers rotate over 4 SWDGE
    # queues (measured ~2.6x faster than a single queue).
    nc = bacc.Bacc("TRN2", num_devices=cfg.NCORES,
                   dynamic_dma_scratch_size=65536, num_swdge_queues=4)
    KQ = cfg.FIN // 128
    HH = cfg.HID // 128
    GW = 128 if cfg.HID > 64 else 128  # g table row width (bf16) -> 256B rows

    xT = nc.dram_tensor("xT", [128, cfg.FIN // 128, cfg.SH], BF16,
                        kind="ExternalInput")
    SC = cfg.SLAB_CH
    w1 = nc.dram_tensor("w1", [128, KQ, cfg.HID], BF16, kind="ExternalInput")
    w2 = nc.dram_tensor("w2", [128, HH, cfg.FOUT], BF16, kind="ExternalInput")
    b1r = nc.dram_tensor("b1r", [1, cfg.HID], BF16, kind="ExternalInput")
    b2r = nc.dram_tensor("b2r", [1, cfg.FOUT], BF16, kind="ExternalInput")
    iota = nc.dram_tensor("iota", [128, SC, 128], BF16, kind="ExternalInput")
    ident = nc.dram_tensor("ident", [128, 128], BF16, kind="ExternalInput")
    onesv = nc.dram_tensor("onesv", [1, 128], BF16, kind="ExternalInput")
    idx_in = nc.dram_tensor("idx", [128, plan.L // 16], I16, kind="ExternalInput")
    dstl_in = nc.dram_tensor("dstl", [128, plan.total_ch], BF16, kind="ExternalInput")
    nrm_in = nc.dram_tensor("nrm", [128, plan.total_ch], BF16, kind="ExternalInput")
    out_ext = nc.dram_tensor("out", [cfg.SH, cfg.FOUT], F32, kind="ExternalOutput")

    groups = [list(range(cfg.NCORES))]

    with tile.TileContext(nc) as tc:
        nc.gpsimd.load_library(mlp_lib)
        tc.no_sync_barrier()
        with (
            tc.tile_pool(name="dram", bufs=1, space="DRAM") as dpool,
            tc.tile_pool(name="const", bufs=1) as cpool,
        ):
            hsh = dpool.tile([cfg.SH, cfg.HID], BF16)
            htab = dpool.tile([cfg.PN, cfg.HID], BF16, addr_space="Shared")
            gsh = dpool.tile([cfg.SH, GW], BF16)
            gtab = dpool.tile([cfg.PN, GW], BF16, addr_space="Shared")

            iota_t = cpool.tile([128, SC, 128], BF16)
            id_t = cpool.tile([128, 128], BF16)
            ones_t = cpool.tile([1, 128], BF16)
            b1_t = cpool.tile([1, cfg.HID], BF16)
            b2_t = cpool.tile([1, cfg.FOUT], BF16)
            w2_t = cpool.tile([128, HH, cfg.FOUT], BF16)
            nc.sync.dma_start(iota_t[:], iota[:])
            nc.sync.dma_start(id_t[:], ident[:])
            nc.sync.dma_start(ones_t[:], onesv[:])
            nc.sync.dma_start(b1_t[:], b1r[:])
            nc.sync.dma_start(b2_t[:], b2r[:])
            nc.sync.dma_start(w2_t[:], w2[:])

            # ---------------- phase 1: h = x @ W1 (shard) ----------------
            with (
                tc.tile_pool(name="p1sb", bufs=1) as p1sb,
                tc.tile_pool(name="p1st", bufs=3) as p1st,
                tc.tile_pool(name="p1ps", bufs=2, space="PSUM") as p1ps,
            ):
                xT_t = p1sb.tile([128, KQ, cfg.SH], BF16)
                w1_t = p1sb.tile([128, KQ, cfg.HID], BF16)
                nc.sync.dma_start(xT_t[:], xT[:])
                nc.sync.dma_start(w1_t[:], w1[:])
                for j in range(cfg.TPS):
                    ph = p1ps.tile([128, cfg.HID], F32, tag="ph")
                    for k in range(KQ):
                        nc.tensor.matmul(
                            ph[:], xT_t[:, k, j * 128:(j + 1) * 128], w1_t[:, k, :],
                            start=(k == 0), stop=(k == KQ - 1))
                    hst = p1st.tile([128, cfg.HID], BF16, tag="hst")
                    nc.scalar.activation(hst[:], ph[:],
                                         mybir.ActivationFunctionType.Copy)
                    nc.sync.dma_start(hsh[j * 128:(j + 1) * 128, :], hst[:])

            # ---------------- phase 2: allgather h ----------------
            nc.gpsimd.collective_compute(
                "AllGather", mybir.AluOpType.bypass, replica_groups=groups,
                ins=[hsh[:].opt()], outs=[htab[:].opt()])

            # num_idxs register constants for gathers (Pool regs are scarce:
            # to_reg doesn't dedupe, so cache handles per distinct value)
            nidx_regs = {}

            def nreg(n):
                if n not in nidx_regs:
                    nidx_regs[n] = nc.gpsimd.to_reg(n)
                return nidx_regs[n]

            # ---------------- phases 3+5 shared aggregation loop ----------
            def agg_pass(tab, elem, out_emit, psum_pool, chunk_emit):
                qn = [0]
                with (
                    tc.tile_pool(name="meta", bufs=2) as mpool,
                    tc.tile_pool(name="gat", bufs=6) as gpool,
                    tc.tile_pool(name="st", bufs=3) as spool,
                ):
                    for b in range(cfg.NB):
                        nch = plan.batch_nch[b]
                        coff = plan.batch_choff[b]
                        idx_b = mpool.tile([128, plan.max_batch_ch * 8], I16, tag="idx")
                        dstl_b = mpool.tile([128, plan.max_batch_ch], BF16, tag="dstl")
                        nrm_b = mpool.tile([128, plan.max_batch_ch], BF16, tag="nrm")
                        nc.sync.dma_start(idx_b[:, :nch * 8],
                                          idx_in[:, coff * 8:(coff + nch) * 8])
                        nc.sync.dma_start(dstl_b[:, :nch], dstl_in[:, coff:coff + nch])
                        nc.sync.dma_start(nrm_b[:, :nch], nrm_in[:, coff:coff + nch])

                        aggs = {}
                        ctiles = plan.batch_chunk_tiles[b]
                        first, last = plan.batch_first[b], plan.batch_last[b]
                        for (w, c0, c1) in plan.batch_slabs[b]:
                            sc = c1 - c0
                            m_t = gpool.tile([128, cfg.SLAB_CH, elem], BF16, tag="m")
                            nc.gpsimd.dma_gather(
                                m_t[:, :sc, :],
                                tab[w * cfg.WIN:(w + 1) * cfg.WIN, :],
                                idx_b[:, c0 * 8:c1 * 8],
                                sc * 128, nreg(sc * 128), elem,
                                queue_num=qn[0])
                            qn[0] = (qn[0] + 1) % 4
                            # one-hot selection matrices for the whole slab in
                            # two wide DVE ops (per-chunk tensor_scalar is
                            # ~10x slower than tensor_tensor on HW)
                            eq_t = spool.tile([128, cfg.SLAB_CH, 128], BF16,
                                              tag="eq")
                            st_t = spool.tile([128, cfg.SLAB_CH, 128], BF16,
                                              tag="st")
                            dl_b = dstl_b[:, c0:c1, None].broadcast_to(
                                (128, sc, 128))
                            nr_b = nrm_b[:, c0:c1, None].broadcast_to(
                                (128, sc, 128))
                            nc.vector.tensor_tensor(
                                eq_t[:, :sc, :], iota_t[:, :sc, :], dl_b,
                                mybir.AluOpType.is_equal)
                            nc.vector.tensor_tensor(
                                st_t[:, :sc, :], eq_t[:, :sc, :], nr_b,
                                mybir.AluOpType.mult)
                            for ci in range(c0, c1):
                                t = ctiles[ci]
                                if t not in aggs:
                                    aggs[t] = psum_pool.tile(
                                        [128, chunk_emit.width], F32,
                                        tag="agg", name=f"agg_b{b}_t{t}")
                                chunk_emit(aggs[t], st_t[:, ci - c0, :],
                                           m_t[:, ci - c0, :],
                                           ci == first[t], ci == last[t])
                        for t in plan.batches[b]:
                            out_emit(t, aggs[t])

            # phase 3 chunk: agg[128d, HID] += S_T.T @ h_rows
            def p3_chunk(agg, st, mrow, is_first, is_last):
                nc.tensor.matmul(agg[:], st[:], mrow, start=is_first, stop=False)
            p3_chunk.width = cfg.HID

            with (
                tc.tile_pool(name="p3ps", bufs=2 * cfg.TB, space="PSUM") as p3ps,
                tc.tile_pool(name="ep", bufs=2) as ep,
                tc.tile_pool(name="eptp", bufs=1, space="PSUM") as eptp,
                tc.tile_pool(name="epg", bufs=1, space="PSUM") as epg,
            ):
                def p3_out(t, agg):
                    # + b1 broadcast, closes the accumulation group
                    nc.tensor.matmul(agg[:], ones_t[:], b1_t[:],
                                     start=False, stop=True)
                    hr = ep.tile([128, cfg.HID], BF16, tag="hr")
                    nc.scalar.activation(hr[:], agg[:],
                                         mybir.ActivationFunctionType.Relu)
                    gp = epg.tile([128, cfg.FOUT], F32, tag="gp")
                    for hh in range(HH):
                        tp = eptp.tile([128, 128], BF16, tag="tp")
                        nc.tensor.transpose(tp[:], hr[:, hh * 128:(hh + 1) * 128],
                                            id_t[:])
                        hrT = ep.tile([128, 128], BF16, tag="hrT")
                        nc.vector.tensor_copy(hrT[:], tp[:])
                        nc.tensor.matmul(gp[:], hrT[:], w2_t[:, hh, :],
                                         start=(hh == 0), stop=(hh == HH - 1))
                    gst = ep.tile([128, GW], BF16, tag="gst")
                    nc.vector.memset(gst[:, cfg.FOUT:], 0.0)
                    nc.scalar.activation(gst[:, :cfg.FOUT], gp[:],
                                         mybir.ActivationFunctionType.Copy)
                    nc.sync.dma_start(gsh[t * 128:(t + 1) * 128, :], gst[:])

                agg_pass(htab, cfg.HID, p3_out, p3ps, p3_chunk)

            # ---------------- phase 4: allgather g ----------------
            nc.gpsimd.collective_compute(
                "AllGather", mybir.AluOpType.bypass, replica_groups=groups,
                ins=[gsh[:].opt()], outs=[gtab[:].opt()])

            # ---------------- phase 5 ----------------
            def p5_chunk(agg, st, mrow, is_first, is_last):
                nc.tensor.matmul(agg[:], st[:], mrow[:, :cfg.FOUT],
                                 start=is_first, stop=False)
            p5_chunk.width = cfg.FOUT

            with (
                tc.tile_pool(name="p5ps", bufs=2 * cfg.TB, space="PSUM") as p5ps,
                tc.tile_pool(name="op", bufs=3) as opool,
            ):
                def p5_out(t, agg):
                    nc.tensor.matmul(agg[:], ones_t[:], b2_t[:],
                                     start=False, stop=True)
                    ost = opool.tile([128, cfg.FOUT], F32, tag="ost")
                    nc.vector.tensor_copy(ost[:], agg[:])
                    nc.sync.dma_start(out_ext[t * 128:(t + 1) * 128, :], ost[:])

                agg_pass(gtab, GW, p5_out, p5ps, p5_chunk)

    nc.compile()
    return nc


def _run(cfg, x, edge_index, edge_weight, W1, b1, W2, b2, **run_kw):
    plan, in_maps = _preprocess(cfg, x, edge_index, edge_weight, W1, b1, W2, b2)
    nc = _build_nc(cfg, plan)
    res = run_bass_kernel_spmd(nc, in_maps, core_ids=list(range(cfg.NCORES)),
                               **run_kw)
    shards = [res.results[c]["out"][:cfg.SH_RAW] for c in range(cfg.NCORES)]
    out = np.concatenate(shards, axis=0).astype(np.float32)
    return out, res


def kernel(x, edge_index, edge_weight, W1, b1, W2, b2):
    cfg = Cfg()
    out, _ = _run(cfg, x, edge_index, edge_weight, W1, b1, W2, b2)
    return out


# revision 25
# speedup vs baseline: 2.1460x; 1.0016x over previous
"""Two-layer GCN (PyG GCNConv semantics) on 8 Trainium2 NeuronCores.

Strategy (graph/data parallel, dst-sharded):
  - Nodes are sharded row-wise across the 8 cores (12500 each, padded to
    12544 = 98*128). Weights are replicated.
  - Phase 1: each core computes its shard of h = x @ W1 (bf16 matmul).
  - Phase 2: AllGather h shards -> full bf16 node-feature table per core.
  - Phase 3: per-core aggregation over incoming edges of its dst shard:
    dma_gather of h[src] rows + one-hot scaled selection matmuls that
    scatter-accumulate into PSUM per 128-dst tile; + b1, ReLU,
    then hr @ W2 (via PE transpose) -> per-node 16-wide g vectors.
  - Phase 4: AllGather g shards -> full g table.
  - Phase 5: same aggregation structure over g rows + b2 -> output shard.

Edge preprocessing (host, numpy): symmetric-norm coefficients, self loops,
edges sorted/grouped per (core, tile-batch, src-window, tile) with
capacities fixed to the max across cores so all 8 cores run one SPMD NEFF.
Index tables are int16 (hardware gather constraint) relative to 25088-row
windows of the padded node table.
"""

import math

import ml_dtypes
import numpy as np

import concourse.bacc as bacc
import concourse.bass as bass
import concourse.mybir as mybir
import concourse.tile as tile
from concourse.bass_utils import run_bass_kernel_spmd
from concourse.library_config import mlp as mlp_lib

F32 = mybir.dt.float32
BF16 = mybir.dt.bfloat16
I16 = mybir.dt.int16
BF = ml_dtypes.bfloat16


class Cfg:
    def __init__(self, N=100000, E=3200000, FIN=512, HID=256, FOUT=16,
                 NCORES=8, TB=3, SLAB_CH=8, MAX_WIN=32768):
        self.N, self.E, self.FIN, self.HID, self.FOUT = N, E, FIN, HID, FOUT
        self.NCORES, self.TB, self.SLAB_CH = NCORES, TB, SLAB_CH
        assert N % NCORES == 0
        self.SH_RAW = N // NCORES
        self.TPS = (self.SH_RAW + 127) // 128          # tiles per shard
        self.SH = self.TPS * 128                        # padded shard rows
        self.PN = NCORES * self.SH                      # padded table rows
        self.W_SH = max(1, min(MAX_WIN // self.SH, NCORES))
        while NCORES % self.W_SH:
            self.W_SH -= 1
        self.WIN = self.W_SH * self.SH                  # window rows (int16-addressable)
        assert self.WIN <= 32768
        self.NWIN = NCORES // self.W_SH
        self.NB = (self.TPS + TB - 1) // TB             # tile batches
        assert FIN % 128 == 0 and HID % 128 == 0 and FOUT <= 128


class Plan:
    """Static (core-independent) edge-stream structure."""

    def __init__(self, cfg, cap):
        # cap: [TPS, NWIN] slot capacity (multiples of 128)
        self.cfg = cfg
        self.cap = cap
        self.batches = [list(range(b * cfg.TB, min((b + 1) * cfg.TB, cfg.TPS)))
                        for b in range(cfg.NB)]
        # per (t, w) chunk offset in the global stream (in chunks of 128 slots)
        self.group_choff = np.zeros((cfg.TPS, cfg.NWIN), np.int64)
        self.batch_choff = []           # chunk offset of each batch
        self.batch_nch = []             # chunks in each batch
        self.batch_slabs = []           # [(w, c0, c1)] chunk ranges (batch-rel)
        self.batch_chunk_tiles = []     # per-chunk tile id
        self.batch_first = []           # tile -> first batch-rel chunk
        self.batch_last = []            # tile -> last batch-rel chunk
        off = 0
        for b, tiles in enumerate(self.batches):
            self.batch_choff.append(off)
            ctiles = []
            slabs = []
            first, last = {}, {}
            for w in range(cfg.NWIN):
                w0 = len(ctiles)
                for t in tiles:
                    self.group_choff[t, w] = off + len(ctiles)
                    nch = cap[t, w] // 128
                    for _ in range(nch):
                        first.setdefault(t, len(ctiles))
                        last[t] = len(ctiles)
                        ctiles.append(t)
                # split into pieces of at most SLAB_CH chunks (each gather's
                # descriptor count must fit the SWDGE ring with headroom;
                # <= SLAB_CH distinct sizes keeps num_idxs register use low)
                c0 = w0
                rem = len(ctiles) - w0
                while rem:
                    p = min(cfg.SLAB_CH, rem)
                    slabs.append((w, c0, c0 + p))
                    c0 += p
                    rem -= p
            self.batch_chunk_tiles.append(ctiles)
            self.batch_slabs.append(slabs)
            self.batch_first.append(first)
            self.batch_last.append(last)
            self.batch_nch.append(len(ctiles))
            off += len(ctiles)
        self.total_ch = off
        self.L = off * 128
        self.max_batch_ch = max(self.batch_nch)


def _preprocess(cfg, x, edge_index, edge_weight, W1, b1, W2, b2):
    N, NC = cfg.N, cfg.NCORES
    src = np.asarray(edge_index[0]).astype(np.int64)
    dst = np.asarray(edge_index[1]).astype(np.int64)
    ew = np.asarray(edge_weight).astype(np.float32)

    # self loops (weight 1.0), symmetric normalization at dst
    deg = np.bincount(dst, weights=ew.astype(np.float64), minlength=N) + 1.0
    dinv = (1.0 / np.sqrt(deg)).astype(np.float32)
    src_f = np.concatenate([src, np.arange(N, dtype=np.int64)])
    dst_f = np.concatenate([dst, np.arange(N, dtype=np.int64)])
    ew_f = np.concatenate([ew, np.ones(N, np.float32)])
    norm = dinv[src_f] * ew_f * dinv[dst_f]

    core = dst_f // cfg.SH_RAW
    dl = dst_f % cfg.SH_RAW
    t = dl // 128
    dtl = (dl % 128).astype(np.float32)
    rsrc = (src_f // cfg.SH_RAW) * cfg.SH + (src_f % cfg.SH_RAW)
    w = rsrc // cfg.WIN
    widx = (rsrc % cfg.WIN).astype(np.int16)

    cnt = np.bincount((core * cfg.TPS + t) * cfg.NWIN + w,
                      minlength=NC * cfg.TPS * cfg.NWIN
                      ).reshape(NC, cfg.TPS, cfg.NWIN)
    cap = ((cnt.max(axis=0) + 127) // 128 * 128).astype(np.int64)
    plan = Plan(cfg, cap)

    # stable sort edges by (core, batch, w, t, widx)
    tb = t // cfg.TB
    order = np.lexsort((widx, t, w, tb, core))
    core_s, t_s, w_s, widx_s = core[order], t[order], w[order], widx[order]
    dtl_s, norm_s = dtl[order], norm[order]
    # rank within (core, t, w) group
    gkey = (core_s * cfg.TPS + t_s) * cfg.NWIN + w_s
    change = np.empty(len(gkey), bool)
    change[0] = True
    change[1:] = gkey[1:] != gkey[:-1]
    gstart = np.flatnonzero(change)
    gsize = np.diff(np.append(gstart, len(gkey)))
    rank = np.arange(len(gkey)) - np.repeat(gstart, gsize)
    pos = plan.group_choff[t_s, w_s] * 128 + rank

    idx16 = np.zeros((NC, plan.L), np.int16)
    dstl = np.full((NC, plan.L), -1.0, np.float32)
    nrm = np.zeros((NC, plan.L), np.float32)
    idx16[core_s, pos] = widx_s
    dstl[core_s, pos] = dtl_s
    nrm[core_s, pos] = norm_s

    # wrapped layouts
    idx_w = idx16.reshape(NC, plan.L // 16, 16).transpose(0, 2, 1)   # [NC,16,L/16]
    idx_w = np.ascontiguousarray(np.tile(idx_w, (1, 8, 1)))          # [NC,128,L/16]
    dstl_w = np.ascontiguousarray(
        dstl.reshape(NC, plan.total_ch, 128).transpose(0, 2, 1).astype(BF))
    nrm_w = np.ascontiguousarray(
        nrm.reshape(NC, plan.total_ch, 128).transpose(0, 2, 1).astype(BF))

    # x^T shards, bf16, zero-padded to SH columns, wrapped [128, FIN//128, SH]
    # with [p, k, n] = x[n, k*128 + p] (matches the SBUF matmul slicing).
    x = np.asarray(x).astype(np.float32)
    KQ = cfg.FIN // 128
    xT = np.zeros((NC, 128, KQ, cfg.SH), BF)
    for c in range(NC):
        xt = x[c * cfg.SH_RAW:(c + 1) * cfg.SH_RAW].T.astype(BF)  # [FIN, SH_RAW]
        xT[c, :, :, :cfg.SH_RAW] = xt.reshape(KQ, 128, cfg.SH_RAW).transpose(1, 0, 2)

    W1 = np.asarray(W1).astype(np.float32)
    W2 = np.asarray(W2).astype(np.float32)
    w1_w = np.ascontiguousarray(
        W1.reshape(cfg.FIN // 128, 128, cfg.HID).transpose(1, 0, 2).astype(BF))
    w2_w = np.ascontiguousarray(
        W2.reshape(cfg.HID // 128, 128, cfg.FOUT).transpose(1, 0, 2).astype(BF))
    b1r = np.asarray(b1).astype(BF).reshape(1, cfg.HID)
    b2r = np.asarray(b2).astype(BF).reshape(1, cfg.FOUT)
    iota = np.tile(np.arange(128, dtype=np.float32)[None, None, :],
                   (128, cfg.SLAB_CH, 1)).astype(BF)
    ident = np.eye(128, dtype=np.float32).astype(BF)
    ones = np.ones((1, 128), BF)

    in_maps = []
    for c in range(NC):
        in_maps.append({
            "xT": np.ascontiguousarray(xT[c]),
            "w1": w1_w, "w2": w2_w, "b1r": b1r, "b2r": b2r,
            "iota": iota, "ident": ident, "onesv": ones,
            "idx": idx_w[c], "dstl": dstl_w[c], "nrm": nrm_w[c],
        })
    return plan, in_maps


def _build_nc(cfg, plan):
    # 64KB/partition SWDGE descriptor carveout; gathers rotate over 4 SWDGE
    # queues (measured ~2.6x faster than a single queue).
    nc = bacc.Bacc("TRN2", num_devices=cfg.NCORES,
                   dynamic_dma_scratch_size=65536, num_swdge_queues=4)
    KQ = cfg.FIN // 128
    HH = cfg.HID // 128
    GW = 128 if cfg.HID > 64 else 128  # g table row width (bf16) -> 256B rows

    xT = nc.dram_tensor("xT", [128, cfg.FIN // 128, cfg.SH], BF16,
                        kind="ExternalInput")
    SC = cfg.SLAB_CH
    w1 = nc.dram_tensor("w1", [128, KQ, cfg.HID], BF16, kind="ExternalInput")
    w2 = nc.dram_tensor("w2", [128, HH, cfg.FOUT], BF16, kind="ExternalInput")
    b1r = nc.dram_tensor("b1r", [1, cfg.HID], BF16, kind="ExternalInput")
    b2r = nc.dram_tensor("b2r", [1, cfg.FOUT], BF16, kind="ExternalInput")
    iota = nc.dram_tensor("iota", [128, SC, 128], BF16, kind="ExternalInput")
    ident = nc.dram_tensor("ident", [128, 128], BF16, kind="ExternalInput")
    onesv = nc.dram_tensor("onesv", [1, 128], BF16, kind="ExternalInput")
    idx_in = nc.dram_tensor("idx", [128, plan.L // 16], I16, kind="ExternalInput")
    dstl_in = nc.dram_tensor("dstl", [128, plan.total_ch], BF16, kind="ExternalInput")
    nrm_in = nc.dram_tensor("nrm", [128, plan.total_ch], BF16, kind="ExternalInput")
    out_ext = nc.dram_tensor("out", [cfg.SH, cfg.FOUT], F32, kind="ExternalOutput")

    groups = [list(range(cfg.NCORES))]

    with tile.TileContext(nc) as tc:
        nc.gpsimd.load_library(mlp_lib)
        tc.no_sync_barrier()
        with (
            tc.tile_pool(name="dram", bufs=1, space="DRAM") as dpool,
            tc.tile_pool(name="const", bufs=1) as cpool,
        ):
            hsh = dpool.tile([cfg.SH, cfg.HID], BF16)
            htab = dpool.tile([cfg.PN, cfg.HID], BF16, addr_space="Shared")
            gsh = dpool.tile([cfg.SH, GW], BF16)
            gtab = dpool.tile([cfg.PN, GW], BF16, addr_space="Shared")

            iota_t = cpool.tile([128, SC, 128], BF16)
            id_t = cpool.tile([128, 128], BF16)
            ones_t = cpool.tile([1, 128], BF16)
            b1_t = cpool.tile([1, cfg.HID], BF16)
            b2_t = cpool.tile([1, cfg.FOUT], BF16)
            w2_t = cpool.tile([128, HH, cfg.FOUT], BF16)
            nc.sync.dma_start(iota_t[:], iota[:])
            nc.sync.dma_start(id_t[:], ident[:])
            nc.sync.dma_start(ones_t[:], onesv[:])
            nc.sync.dma_start(b1_t[:], b1r[:])
            nc.sync.dma_start(b2_t[:], b2r[:])
            nc.sync.dma_start(w2_t[:], w2[:])

            # ---------------- phase 1: h = x @ W1 (shard) ----------------
            with (
                tc.tile_pool(name="p1sb", bufs=1) as p1sb,
                tc.tile_pool(name="p1st", bufs=3) as p1st,
                tc.tile_pool(name="p1ps", bufs=2, space="PSUM") as p1ps,
            ):
                xT_t = p1sb.tile([128, KQ, cfg.SH], BF16)
                w1_t = p1sb.tile([128, KQ, cfg.HID], BF16)
                nc.sync.dma_start(xT_t[:], xT[:])
                nc.sync.dma_start(w1_t[:], w1[:])
                for j in range(cfg.TPS):
                    ph = p1ps.tile([128, cfg.HID], F32, tag="ph")
                    for k in range(KQ):
                        nc.tensor.matmul(
                            ph[:], xT_t[:, k, j * 128:(j + 1) * 128], w1_t[:, k, :],
                            start=(k == 0), stop=(k == KQ - 1))
                    hst = p1st.tile([128, cfg.HID], BF16, tag="hst")
                    nc.scalar.activation(hst[:], ph[:],
                                         mybir.ActivationFunctionType.Copy)
                    nc.sync.dma_start(hsh[j * 128:(j + 1) * 128, :], hst[:])

            # ---------------- phase 2: allgather h ----------------
            nc.gpsimd.collective_compute(
                "AllGather", mybir.AluOpType.bypass, replica_groups=groups,
                ins=[hsh[:].opt()], outs=[htab[:].opt()])

            # num_idxs register constants for gathers (Pool regs are scarce:
            # to_reg doesn't dedupe, so cache handles per distinct value)
            nidx_regs = {}

            def nreg(n):
                if n not in nidx_regs:
                    nidx_regs[n] = nc.gpsimd.to_reg(n)
                return nidx_regs[n]

            # ---------------- phases 3+5 shared aggregation loop ----------
            def agg_pass(tab, elem, out_emit, psum_pool, chunk_emit):
                qn = [0]
                with (
                    tc.tile_pool(name="meta", bufs=3) as mpool,
                    tc.tile_pool(name="gat", bufs=8) as gpool,
                    tc.tile_pool(name="st", bufs=3) as spool,
                ):
                    for b in range(cfg.NB):
                        nch = plan.batch_nch[b]
                        coff = plan.batch_choff[b]
                        idx_b = mpool.tile([128, plan.max_batch_ch * 8], I16, tag="idx")
                        dstl_b = mpool.tile([128, plan.max_batch_ch], BF16, tag="dstl")
                        nrm_b = mpool.tile([128, plan.max_batch_ch], BF16, tag="nrm")
                        nc.sync.dma_start(idx_b[:, :nch * 8],
                                          idx_in[:, coff * 8:(coff + nch) * 8])
                        nc.sync.dma_start(dstl_b[:, :nch], dstl_in[:, coff:coff + nch])
                        nc.sync.dma_start(nrm_b[:, :nch], nrm_in[:, coff:coff + nch])

                        aggs = {}
                        ctiles = plan.batch_chunk_tiles[b]
                        first, last = plan.batch_first[b], plan.batch_last[b]
                        for (w, c0, c1) in plan.batch_slabs[b]:
                            sc = c1 - c0
                            m_t = gpool.tile([128, cfg.SLAB_CH, elem], BF16, tag="m")
                            nc.gpsimd.dma_gather(
                                m_t[:, :sc, :],
                                tab[w * cfg.WIN:(w + 1) * cfg.WIN, :],
                                idx_b[:, c0 * 8:c1 * 8],
                                sc * 128, nreg(sc * 128), elem,
                                single_packet=False, queue_num=qn[0])
                            qn[0] = (qn[0] + 1) % 4
                            # one-hot selection matrices for the whole slab in
                            # two wide DVE ops (per-chunk tensor_scalar is
                            # ~10x slower than tensor_tensor on HW)
                            eq_t = spool.tile([128, cfg.SLAB_CH, 128], BF16,
                                              tag="eq")
                            st_t = spool.tile([128, cfg.SLAB_CH, 128], BF16,
                                              tag="st")
                            dl_b = dstl_b[:, c0:c1, None].broadcast_to(
                                (128, sc, 128))
                            nr_b = nrm_b[:, c0:c1, None].broadcast_to(
                                (128, sc, 128))
                            nc.vector.tensor_tensor(
                                eq_t[:, :sc, :], iota_t[:, :sc, :], dl_b,
                                mybir.AluOpType.is_equal)
                            nc.vector.tensor_tensor(
                                st_t[:, :sc, :], eq_t[:, :sc, :], nr_b,
                                mybir.AluOpType.mult)
                            for ci in range(c0, c1):
                                t = ctiles[ci]
                                if t not in aggs:
                                    aggs[t] = psum_pool.tile(
                                        [128, chunk_emit.width], F32,
                                        tag="agg", name=f"agg_b{b}_t{t}")
                                chunk_emit(aggs[t], st_t[:, ci - c0, :],
                                           m_t[:, ci - c0, :],
                                           ci == first[t], ci == last[t])
                        for t in plan.batches[b]:
                            out_emit(t, aggs[t])

            # phase 3 chunk: agg[128d, HID] += S_T.T @ h_rows
            def p3_chunk(agg, st, mrow, is_first, is_last):
                nc.tensor.matmul(agg[:], st[:], mrow, start=is_first, stop=False)
            p3_chunk.width = cfg.HID

            with (
                tc.tile_pool(name="p3ps", bufs=2 * cfg.TB, space="PSUM") as p3ps,
                tc.tile_pool(name="ep", bufs=2) as ep,
                tc.tile_pool(name="eptp", bufs=1, space="PSUM") as eptp,
                tc.tile_pool(name="epg", bufs=1, space="PSUM") as epg,
            ):
                def p3_out(t, agg):
                    # + b1 broadcast, closes the accumulation group
                    nc.tensor.matmul(agg[:], ones_t[:], b1_t[:],
                                     start=False, stop=True)
                    hr = ep.tile([128, cfg.HID], BF16, tag="hr")
                    nc.scalar.activation(hr[:], agg[:],
                                         mybir.ActivationFunctionType.Relu)
                    gp = epg.tile([128, cfg.FOUT], F32, tag="gp")
                    for hh in range(HH):
                        tp = eptp.tile([128, 128], BF16, tag="tp")
                        nc.tensor.transpose(tp[:], hr[:, hh * 128:(hh + 1) * 128],
                                            id_t[:])
                        hrT = ep.tile([128, 128], BF16, tag="hrT")
                        nc.vector.tensor_copy(hrT[:], tp[:])
                        nc.tensor.matmul(gp[:], hrT[:], w2_t[:, hh, :],
                                         start=(hh == 0), stop=(hh == HH - 1))
                    gst = ep.tile([128, GW], BF16, tag="gst")
                    nc.vector.memset(gst[:, cfg.FOUT:], 0.0)
                    nc.scalar.activation(gst[:, :cfg.FOUT], gp[:],
                                         mybir.ActivationFunctionType.Copy)
                    nc.sync.dma_start(gsh[t * 128:(t + 1) * 128, :], gst[:])

                agg_pass(htab, cfg.HID, p3_out, p3ps, p3_chunk)

            # ---------------- phase 4: allgather g ----------------
            nc.gpsimd.collective_compute(
                "AllGather", mybir.AluOpType.bypass, replica_groups=groups,
                ins=[gsh[:].opt()], outs=[gtab[:].opt()])

            # ---------------- phase 5 ----------------
            def p5_chunk(agg, st, mrow, is_first, is_last):
                nc.tensor.matmul(agg[:], st[:], mrow[:, :cfg.FOUT],
                                 start=is_first, stop=False)
            p5_chunk.width = cfg.FOUT

            with (
                tc.tile_pool(name="p5ps", bufs=2 * cfg.TB, space="PSUM") as p5ps,
                tc.tile_pool(name="op", bufs=3) as opool,
            ):
                def p5_out(t, agg):
                    nc.tensor.matmul(agg[:], ones_t[:], b2_t[:],
                                     start=False, stop=True)
                    ost = opool.tile([128, cfg.FOUT], F32, tag="ost")
                    nc.vector.tensor_copy(ost[:], agg[:])
                    nc.sync.dma_start(out_ext[t * 128:(t + 1) * 128, :], ost[:])

                agg_pass(gtab, GW, p5_out, p5ps, p5_chunk)

    nc.compile()
    return nc


def _run(cfg, x, edge_index, edge_weight, W1, b1, W2, b2, **run_kw):
    plan, in_maps = _preprocess(cfg, x, edge_index, edge_weight, W1, b1, W2, b2)
    nc = _build_nc(cfg, plan)
    res = run_bass_kernel_spmd(nc, in_maps, core_ids=list(range(cfg.NCORES)),
                               **run_kw)
    shards = [res.results[c]["out"][:cfg.SH_RAW] for c in range(cfg.NCORES)]
    out = np.concatenate(shards, axis=0).astype(np.float32)
    return out, res


def kernel(x, edge_index, edge_weight, W1, b1, W2, b2):
    cfg = Cfg()
    out, _ = _run(cfg, x, edge_index, edge_weight, W1, b1, W2, b2)
    return out


# revision 26
# speedup vs baseline: 2.4292x; 1.1319x over previous
"""Two-layer GCN (PyG GCNConv semantics) on 8 Trainium2 NeuronCores.

Strategy (graph/data parallel, dst-sharded):
  - Nodes are sharded row-wise across the 8 cores (12500 each, padded to
    12544 = 98*128). Weights are replicated.
  - Phase 1: each core computes its shard of h = x @ W1 (bf16 matmul).
  - Phase 2: AllGather h shards -> full bf16 node-feature table per core.
  - Phase 3: per-core aggregation over incoming edges of its dst shard:
    dma_gather of h[src] rows + one-hot scaled selection matmuls that
    scatter-accumulate into PSUM per 128-dst tile; + b1, ReLU,
    then hr @ W2 (via PE transpose) -> per-node 16-wide g vectors.
  - Phase 4: AllGather g shards -> full g table.
  - Phase 5: same aggregation structure over g rows + b2 -> output shard.

Edge preprocessing (host, numpy): symmetric-norm coefficients, self loops,
edges sorted/grouped per (core, tile-batch, src-window, tile) with
capacities fixed to the max across cores so all 8 cores run one SPMD NEFF.
Index tables are int16 (hardware gather constraint) relative to 25088-row
windows of the padded node table.
"""

import math

import ml_dtypes
import numpy as np

import concourse.bacc as bacc
import concourse.bass as bass
import concourse.mybir as mybir
import concourse.tile as tile
from concourse.bass_utils import run_bass_kernel_spmd
from concourse.library_config import mlp as mlp_lib

F32 = mybir.dt.float32
BF16 = mybir.dt.bfloat16
I16 = mybir.dt.int16
BF = ml_dtypes.bfloat16


class Cfg:
    def __init__(self, N=100000, E=3200000, FIN=512, HID=256, FOUT=16,
                 NCORES=8, TB=3, SLAB_CH=10, MAX_WIN=32768):
        self.N, self.E, self.FIN, self.HID, self.FOUT = N, E, FIN, HID, FOUT
        self.NCORES, self.TB, self.SLAB_CH = NCORES, TB, SLAB_CH
        assert N % NCORES == 0
        self.SH_RAW = N // NCORES
        self.TPS = (self.SH_RAW + 127) // 128          # tiles per shard
        self.SH = self.TPS * 128                        # padded shard rows
        self.PN = NCORES * self.SH                      # padded table rows
        self.W_SH = max(1, min(MAX_WIN // self.SH, NCORES))
        while NCORES % self.W_SH:
            self.W_SH -= 1
        self.WIN = self.W_SH * self.SH                  # window rows (int16-addressable)
        assert self.WIN <= 32768
        self.NWIN = NCORES // self.W_SH
        self.NB = (self.TPS + TB - 1) // TB             # tile batches
        assert FIN % 128 == 0 and HID % 128 == 0 and FOUT <= 128


class Plan:
    """Static (core-independent) edge-stream structure."""

    def __init__(self, cfg, cap):
        # cap: [TPS, NWIN] slot capacity (multiples of 128)
        self.cfg = cfg
        self.cap = cap
        self.batches = [list(range(b * cfg.TB, min((b + 1) * cfg.TB, cfg.TPS)))
                        for b in range(cfg.NB)]
        # per (t, w) chunk offset in the global stream (in chunks of 128 slots)
        self.group_choff = np.zeros((cfg.TPS, cfg.NWIN), np.int64)
        self.batch_choff = []           # chunk offset of each batch
        self.batch_nch = []             # chunks in each batch
        self.batch_slabs = []           # [(w, c0, c1)] chunk ranges (batch-rel)
        self.batch_chunk_tiles = []     # per-chunk tile id
        self.batch_first = []           # tile -> first batch-rel chunk
        self.batch_last = []            # tile -> last batch-rel chunk
        off = 0
        for b, tiles in enumerate(self.batches):
            self.batch_choff.append(off)
            ctiles = []
            slabs = []
            first, last = {}, {}
            for w in range(cfg.NWIN):
                w0 = len(ctiles)
                for t in tiles:
                    self.group_choff[t, w] = off + len(ctiles)
                    nch = cap[t, w] // 128
                    for _ in range(nch):
                        first.setdefault(t, len(ctiles))
                        last[t] = len(ctiles)
                        ctiles.append(t)
                # split into pieces of at most SLAB_CH chunks (each gather's
                # descriptor count must fit the SWDGE ring with headroom;
                # <= SLAB_CH distinct sizes keeps num_idxs register use low)
                c0 = w0
                rem = len(ctiles) - w0
                while rem:
                    p = min(cfg.SLAB_CH, rem)
                    slabs.append((w, c0, c0 + p))
                    c0 += p
                    rem -= p
            self.batch_chunk_tiles.append(ctiles)
            self.batch_slabs.append(slabs)
            self.batch_first.append(first)
            self.batch_last.append(last)
            self.batch_nch.append(len(ctiles))
            off += len(ctiles)
        self.total_ch = off
        self.L = off * 128
        self.max_batch_ch = max(self.batch_nch)


def _preprocess(cfg, x, edge_index, edge_weight, W1, b1, W2, b2):
    N, NC = cfg.N, cfg.NCORES
    src = np.asarray(edge_index[0]).astype(np.int64)
    dst = np.asarray(edge_index[1]).astype(np.int64)
    ew = np.asarray(edge_weight).astype(np.float32)

    # self loops (weight 1.0), symmetric normalization at dst
    deg = np.bincount(dst, weights=ew.astype(np.float64), minlength=N) + 1.0
    dinv = (1.0 / np.sqrt(deg)).astype(np.float32)
    src_f = np.concatenate([src, np.arange(N, dtype=np.int64)])
    dst_f = np.concatenate([dst, np.arange(N, dtype=np.int64)])
    ew_f = np.concatenate([ew, np.ones(N, np.float32)])
    norm = dinv[src_f] * ew_f * dinv[dst_f]

    core = dst_f // cfg.SH_RAW
    dl = dst_f % cfg.SH_RAW
    t = dl // 128
    dtl = (dl % 128).astype(np.float32)
    rsrc = (src_f // cfg.SH_RAW) * cfg.SH + (src_f % cfg.SH_RAW)
    w = rsrc // cfg.WIN
    widx = (rsrc % cfg.WIN).astype(np.int16)

    cnt = np.bincount((core * cfg.TPS + t) * cfg.NWIN + w,
                      minlength=NC * cfg.TPS * cfg.NWIN
                      ).reshape(NC, cfg.TPS, cfg.NWIN)
    cap = ((cnt.max(axis=0) + 127) // 128 * 128).astype(np.int64)
    plan = Plan(cfg, cap)

    # stable sort edges by (core, batch, w, t, widx)
    tb = t // cfg.TB
    order = np.lexsort((widx, t, w, tb, core))
    core_s, t_s, w_s, widx_s = core[order], t[order], w[order], widx[order]
    dtl_s, norm_s = dtl[order], norm[order]
    # rank within (core, t, w) group
    gkey = (core_s * cfg.TPS + t_s) * cfg.NWIN + w_s
    change = np.empty(len(gkey), bool)
    change[0] = True
    change[1:] = gkey[1:] != gkey[:-1]
    gstart = np.flatnonzero(change)
    gsize = np.diff(np.append(gstart, len(gkey)))
    rank = np.arange(len(gkey)) - np.repeat(gstart, gsize)
    pos = plan.group_choff[t_s, w_s] * 128 + rank

    idx16 = np.zeros((NC, plan.L), np.int16)
    dstl = np.full((NC, plan.L), -1.0, np.float32)
    nrm = np.zeros((NC, plan.L), np.float32)
    idx16[core_s, pos] = widx_s
    dstl[core_s, pos] = dtl_s
    nrm[core_s, pos] = norm_s

    # wrapped layouts
    idx_w = idx16.reshape(NC, plan.L // 16, 16).transpose(0, 2, 1)   # [NC,16,L/16]
    idx_w = np.ascontiguousarray(np.tile(idx_w, (1, 8, 1)))          # [NC,128,L/16]
    dstl_w = np.ascontiguousarray(
        dstl.reshape(NC, plan.total_ch, 128).transpose(0, 2, 1).astype(BF))
    nrm_w = np.ascontiguousarray(
        nrm.reshape(NC, plan.total_ch, 128).transpose(0, 2, 1).astype(BF))

    # x^T shards, bf16, zero-padded to SH columns, wrapped [128, FIN//128, SH]
    # with [p, k, n] = x[n, k*128 + p] (matches the SBUF matmul slicing).
    x = np.asarray(x).astype(np.float32)
    KQ = cfg.FIN // 128
    xT = np.zeros((NC, 128, KQ, cfg.SH), BF)
    for c in range(NC):
        xt = x[c * cfg.SH_RAW:(c + 1) * cfg.SH_RAW].T.astype(BF)  # [FIN, SH_RAW]
        xT[c, :, :, :cfg.SH_RAW] = xt.reshape(KQ, 128, cfg.SH_RAW).transpose(1, 0, 2)

    W1 = np.asarray(W1).astype(np.float32)
    W2 = np.asarray(W2).astype(np.float32)
    w1_w = np.ascontiguousarray(
        W1.reshape(cfg.FIN // 128, 128, cfg.HID).transpose(1, 0, 2).astype(BF))
    w2_w = np.ascontiguousarray(
        W2.reshape(cfg.HID // 128, 128, cfg.FOUT).transpose(1, 0, 2).astype(BF))
    b1r = np.asarray(b1).astype(BF).reshape(1, cfg.HID)
    b2r = np.asarray(b2).astype(BF).reshape(1, cfg.FOUT)
    iota = np.tile(np.arange(128, dtype=np.float32)[None, None, :],
                   (128, cfg.SLAB_CH, 1)).astype(BF)
    ident = np.eye(128, dtype=np.float32).astype(BF)
    ones = np.ones((1, 128), BF)

    in_maps = []
    for c in range(NC):
        in_maps.append({
            "xT": np.ascontiguousarray(xT[c]),
            "w1": w1_w, "w2": w2_w, "b1r": b1r, "b2r": b2r,
            "iota": iota, "ident": ident, "onesv": ones,
            "idx": idx_w[c], "dstl": dstl_w[c], "nrm": nrm_w[c],
        })
    return plan, in_maps


def _build_nc(cfg, plan):
    # 64KB/partition SWDGE descriptor carveout; gathers rotate over 4 SWDGE
    # queues (measured ~2.6x faster than a single queue).
    nc = bacc.Bacc("TRN2", num_devices=cfg.NCORES,
                   dynamic_dma_scratch_size=65536, num_swdge_queues=4)
    KQ = cfg.FIN // 128
    HH = cfg.HID // 128
    GW = 128 if cfg.HID > 64 else 128  # g table row width (bf16) -> 256B rows

    xT = nc.dram_tensor("xT", [128, cfg.FIN // 128, cfg.SH], BF16,
                        kind="ExternalInput")
    SC = cfg.SLAB_CH
    w1 = nc.dram_tensor("w1", [128, KQ, cfg.HID], BF16, kind="ExternalInput")
    w2 = nc.dram_tensor("w2", [128, HH, cfg.FOUT], BF16, kind="ExternalInput")
    b1r = nc.dram_tensor("b1r", [1, cfg.HID], BF16, kind="ExternalInput")
    b2r = nc.dram_tensor("b2r", [1, cfg.FOUT], BF16, kind="ExternalInput")
    iota = nc.dram_tensor("iota", [128, SC, 128], BF16, kind="ExternalInput")
    ident = nc.dram_tensor("ident", [128, 128], BF16, kind="ExternalInput")
    onesv = nc.dram_tensor("onesv", [1, 128], BF16, kind="ExternalInput")
    idx_in = nc.dram_tensor("idx", [128, plan.L // 16], I16, kind="ExternalInput")
    dstl_in = nc.dram_tensor("dstl", [128, plan.total_ch], BF16, kind="ExternalInput")
    nrm_in = nc.dram_tensor("nrm", [128, plan.total_ch], BF16, kind="ExternalInput")
    out_ext = nc.dram_tensor("out", [cfg.SH, cfg.FOUT], F32, kind="ExternalOutput")

    groups = [list(range(cfg.NCORES))]

    with tile.TileContext(nc) as tc:
        nc.gpsimd.load_library(mlp_lib)
        tc.no_sync_barrier()
        with (
            tc.tile_pool(name="dram", bufs=1, space="DRAM") as dpool,
            tc.tile_pool(name="const", bufs=1) as cpool,
        ):
            hsh = dpool.tile([cfg.SH, cfg.HID], BF16)
            htab = dpool.tile([cfg.PN, cfg.HID], BF16, addr_space="Shared")
            gsh = dpool.tile([cfg.SH, GW], BF16)
            gtab = dpool.tile([cfg.PN, GW], BF16, addr_space="Shared")

            iota_t = cpool.tile([128, SC, 128], BF16)
            id_t = cpool.tile([128, 128], BF16)
            ones_t = cpool.tile([1, 128], BF16)
            b1_t = cpool.tile([1, cfg.HID], BF16)
            b2_t = cpool.tile([1, cfg.FOUT], BF16)
            w2_t = cpool.tile([128, HH, cfg.FOUT], BF16)
            nc.sync.dma_start(iota_t[:], iota[:])
            nc.sync.dma_start(id_t[:], ident[:])
            nc.sync.dma_start(ones_t[:], onesv[:])
            nc.sync.dma_start(b1_t[:], b1r[:])
            nc.sync.dma_start(b2_t[:], b2r[:])
            nc.sync.dma_start(w2_t[:], w2[:])

            # ---------------- phase 1: h = x @ W1 (shard) ----------------
            with (
                tc.tile_pool(name="p1sb", bufs=1) as p1sb,
                tc.tile_pool(name="p1st", bufs=3) as p1st,
                tc.tile_pool(name="p1ps", bufs=2, space="PSUM") as p1ps,
            ):
                xT_t = p1sb.tile([128, KQ, cfg.SH], BF16)
                w1_t = p1sb.tile([128, KQ, cfg.HID], BF16)
                nc.sync.dma_start(xT_t[:], xT[:])
                nc.sync.dma_start(w1_t[:], w1[:])
                for j in range(cfg.TPS):
                    ph = p1ps.tile([128, cfg.HID], F32, tag="ph")
                    for k in range(KQ):
                        nc.tensor.matmul(
                            ph[:], xT_t[:, k, j * 128:(j + 1) * 128], w1_t[:, k, :],
                            start=(k == 0), stop=(k == KQ - 1))
                    hst = p1st.tile([128, cfg.HID], BF16, tag="hst")
                    nc.scalar.activation(hst[:], ph[:],
                                         mybir.ActivationFunctionType.Copy)
                    nc.sync.dma_start(hsh[j * 128:(j + 1) * 128, :], hst[:])

            # ---------------- phase 2: allgather h ----------------
            nc.gpsimd.collective_compute(
                "AllGather", mybir.AluOpType.bypass, replica_groups=groups,
                ins=[hsh[:].opt()], outs=[htab[:].opt()])

            # num_idxs register constants for gathers (Pool regs are scarce:
            # to_reg doesn't dedupe, so cache handles per distinct value)
            nidx_regs = {}

            def nreg(n):
                if n not in nidx_regs:
                    nidx_regs[n] = nc.gpsimd.to_reg(n)
                return nidx_regs[n]

            # ---------------- phases 3+5 shared aggregation loop ----------
            def agg_pass(tab, elem, out_emit, psum_pool, chunk_emit):
                qn = [0]
                with (
                    tc.tile_pool(name="meta", bufs=3) as mpool,
                    tc.tile_pool(name="gat", bufs=8) as gpool,
                    tc.tile_pool(name="st", bufs=3) as spool,
                ):
                    for b in range(cfg.NB):
                        nch = plan.batch_nch[b]
                        coff = plan.batch_choff[b]
                        idx_b = mpool.tile([128, plan.max_batch_ch * 8], I16, tag="idx")
                        dstl_b = mpool.tile([128, plan.max_batch_ch], BF16, tag="dstl")
                        nrm_b = mpool.tile([128, plan.max_batch_ch], BF16, tag="nrm")
                        nc.sync.dma_start(idx_b[:, :nch * 8],
                                          idx_in[:, coff * 8:(coff + nch) * 8])
                        nc.sync.dma_start(dstl_b[:, :nch], dstl_in[:, coff:coff + nch])
                        nc.sync.dma_start(nrm_b[:, :nch], nrm_in[:, coff:coff + nch])

                        aggs = {}
                        ctiles = plan.batch_chunk_tiles[b]
                        first, last = plan.batch_first[b], plan.batch_last[b]
                        for (w, c0, c1) in plan.batch_slabs[b]:
                            sc = c1 - c0
                            m_t = gpool.tile([128, cfg.SLAB_CH, elem], BF16, tag="m")
                            nc.gpsimd.dma_gather(
                                m_t[:, :sc, :],
                                tab[w * cfg.WIN:(w + 1) * cfg.WIN, :],
                                idx_b[:, c0 * 8:c1 * 8],
                                sc * 128, nreg(sc * 128), elem,
                                single_packet=False, queue_num=qn[0])
                            qn[0] = (qn[0] + 1) % 4
                            # one-hot selection matrices for the whole slab in
                            # two wide DVE ops (per-chunk tensor_scalar is
                            # ~10x slower than tensor_tensor on HW)
                            eq_t = spool.tile([128, cfg.SLAB_CH, 128], BF16,
                                              tag="eq")
                            st_t = spool.tile([128, cfg.SLAB_CH, 128], BF16,
                                              tag="st")
                            dl_b = dstl_b[:, c0:c1, None].broadcast_to(
                                (128, sc, 128))
                            nr_b = nrm_b[:, c0:c1, None].broadcast_to(
                                (128, sc, 128))
                            nc.vector.tensor_tensor(
                                eq_t[:, :sc, :], iota_t[:, :sc, :], dl_b,
                                mybir.AluOpType.is_equal)
                            nc.vector.tensor_tensor(
                                st_t[:, :sc, :], eq_t[:, :sc, :], nr_b,
                                mybir.AluOpType.mult)
                            for ci in range(c0, c1):
                                t = ctiles[ci]
                                if t not in aggs:
                                    aggs[t] = psum_pool.tile(
                                        [128, chunk_emit.width], F32,
                                        tag="agg", name=f"agg_b{b}_t{t}")
                                chunk_emit(aggs[t], st_t[:, ci - c0, :],
                                           m_t[:, ci - c0, :],
                                           ci == first[t], ci == last[t])
                        for t in plan.batches[b]:
                            out_emit(t, aggs[t])

            # phase 3 chunk: agg[128d, HID] += S_T.T @ h_rows
            def p3_chunk(agg, st, mrow, is_first, is_last):
                nc.tensor.matmul(agg[:], st[:], mrow, start=is_first, stop=False)
            p3_chunk.width = cfg.HID

            with (
                tc.tile_pool(name="p3ps", bufs=2 * cfg.TB, space="PSUM") as p3ps,
                tc.tile_pool(name="ep", bufs=2) as ep,
                tc.tile_pool(name="eptp", bufs=1, space="PSUM") as eptp,
                tc.tile_pool(name="epg", bufs=1, space="PSUM") as epg,
            ):
                def p3_out(t, agg):
                    # + b1 broadcast, closes the accumulation group
                    nc.tensor.matmul(agg[:], ones_t[:], b1_t[:],
                                     start=False, stop=True)
                    hr = ep.tile([128, cfg.HID], BF16, tag="hr")
                    nc.scalar.activation(hr[:], agg[:],
                                         mybir.ActivationFunctionType.Relu)
                    gp = epg.tile([128, cfg.FOUT], F32, tag="gp")
                    for hh in range(HH):
                        tp = eptp.tile([128, 128], BF16, tag="tp")
                        nc.tensor.transpose(tp[:], hr[:, hh * 128:(hh + 1) * 128],
                                            id_t[:])
                        hrT = ep.tile([128, 128], BF16, tag="hrT")
                        nc.vector.tensor_copy(hrT[:], tp[:])
                        nc.tensor.matmul(gp[:], hrT[:], w2_t[:, hh, :],
                                         start=(hh == 0), stop=(hh == HH - 1))
                    gst = ep.tile([128, GW], BF16, tag="gst")
                    nc.vector.memset(gst[:, cfg.FOUT:], 0.0)
                    nc.scalar.activation(gst[:, :cfg.FOUT], gp[:],
                                         mybir.ActivationFunctionType.Copy)
                    nc.sync.dma_start(gsh[t * 128:(t + 1) * 128, :], gst[:])

                agg_pass(htab, cfg.HID, p3_out, p3ps, p3_chunk)

            # ---------------- phase 4: allgather g ----------------
            nc.gpsimd.collective_compute(
                "AllGather", mybir.AluOpType.bypass, replica_groups=groups,
                ins=[gsh[:].opt()], outs=[gtab[:].opt()])

            # ---------------- phase 5 ----------------
            def p5_chunk(agg, st, mrow, is_first, is_last):
                nc.tensor.matmul(agg[:], st[:], mrow[:, :cfg.FOUT],
                                 start=is_first, stop=False)
            p5_chunk.width = cfg.FOUT

            with (
                tc.tile_pool(name="p5ps", bufs=2 * cfg.TB, space="PSUM") as p5ps,
                tc.tile_pool(name="op", bufs=3) as opool,
            ):
                def p5_out(t, agg):
                    nc.tensor.matmul(agg[:], ones_t[:], b2_t[:],
                                     start=False, stop=True)
                    ost = opool.tile([128, cfg.FOUT], F32, tag="ost")
                    nc.vector.tensor_copy(ost[:], agg[:])
                    nc.sync.dma_start(out_ext[t * 128:(t + 1) * 128, :], ost[:])

                agg_pass(gtab, GW, p5_out, p5ps, p5_chunk)

    nc.compile()
    return nc


def _run(cfg, x, edge_index, edge_weight, W1, b1, W2, b2, **run_kw):
    plan, in_maps = _preprocess(cfg, x, edge_index, edge_weight, W1, b1, W2, b2)
    nc = _build_nc(cfg, plan)
    res = run_bass_kernel_spmd(nc, in_maps, core_ids=list(range(cfg.NCORES)),
                               **run_kw)
    shards = [res.results[c]["out"][:cfg.SH_RAW] for c in range(cfg.NCORES)]
    out = np.concatenate(shards, axis=0).astype(np.float32)
    return out, res


def kernel(x, edge_index, edge_weight, W1, b1, W2, b2):
    cfg = Cfg()
    out, _ = _run(cfg, x, edge_index, edge_weight, W1, b1, W2, b2)
    return out
